# revision 23
# baseline (speedup 1.0000x reference)
"""3-layer GCN (GCNConv x3, tanh between) on 8 Trainium2 NeuronCores.

Strategy (v2 — "SpMM-first" restructure of the node-range-sharded scheme):
  - GCN aggregation commutes with the dense transform (both linear), so
    layer 1 aggregates the *input features* x directly: each core holds
    the full bf16 copy of x in DRAM as a gather table, so layer-1 message
    gathering starts at t=0 with no AllGather and 256-wide (not 512-wide)
    rows. The aggregated block is then densified locally:
        z1 = tanh(aggT_x^T @ W1 + b1).
  - Layer 2 is also SpMM-first on the AllGathered z1 table. The chunk
    matmuls are emitted transposed (aggT_k = G_k^T @ S, [fin_chunk, dst])
    so the aggregate lands feature-major and feeds the dense matmul's
    stationary operand directly — no transposes for z1/z2-in.
  - Layer 3 is dense-first (h3 = z2 @ W3 is 256-wide, halving both its
    AllGather and its gather traffic); z2 is transposed on the
    TensorEngine (bf16, 1 cyc/row) to feature-major for that matmul.
  - Edges (+ one self-edge per node, weight dinv^2) are bucketed per
    128-dst-node block and *deduplicated by src* within the block
    (~10% fewer gathered rows and chunks); the per-chunk S matrix
    [row, dst_local] accumulates duplicate edge weights. One S table
    serves all three layers. Gathers use the GPSIMD dma_gather extended
    instruction batched 8 chunks / 1024 rows per instruction.
  - All matmuls are bf16 (1 cycle/row on the PE) accumulating fp32 in
    PSUM; biases enter PSUM as a rank-1 ones^T @ b matmul.

Numerics: messages, aggregates, and weights are bf16; accumulation is
fp32. End-to-end relative L2 error vs the fp32 reference is ~5e-3.
Host preprocessing touches only edge_index (sorting/bincount/unique),
the degree-derived edge weights, and bf16 casts of x/W/b.
"""
import sys

if "/opt/trn_rl_repo" not in sys.path:
    sys.path.insert(0, "/opt/trn_rl_repo")

from contextlib import ExitStack

import ml_dtypes
import numpy as np

import concourse.bass as bass
import concourse.bacc as bacc
import concourse.mybir as mybir
import concourse.tile as tile
from concourse.bass_utils import run_bass_kernel_spmd
from concourse.masks import make_identity

P = 128
N_CORES = 8
N_NODES = 10000
SHARD = N_NODES // N_CORES          # 1250
N_BLOCKS = (SHARD + P - 1) // P     # 10 (9 full + one 98-row block)
IN_DIM, HID_DIM, OUT_DIM = 256, 512, 256
SA, SB = 640, SHARD - 640           # split-AllGather half sizes
GK = 8                              # gather chunks per dma_gather instr

_DT = mybir.dt.float32
_DTG = mybir.dt.bfloat16

_BF = ml_dtypes.bfloat16


# ----------------------------------------------------------------------------
# Host-side edge preprocessing
# ----------------------------------------------------------------------------

def _preprocess(edge_index: np.ndarray):
    """Bucket edges by dst block, dedup srcs per block, build S + gidx.

    Within each block the deduped src rows are split by AllGather half
    (set0: q < SA, landing in hf[:8*SA]; set1: q >= SA) so the device can
    gather set0 rows as soon as AG half-0 completes, overlapping half-1's
    collective latency with gather traffic.

    Returns (schedule, gidx1_pc, gidxA_pc, s_pc):
      schedule    : (sched0, sched1) per-block chunk counts (all cores)
      gidx1_pc    : [P, C*8] int16 per core, plain node-id gather indices
                    (for the replicated x table)
      gidxA_pc    : [P, C*8] int16 per core, AG-layout indices; set1 rows
                    are RELATIVE to hf[8*SA:] (gathers use the half-table
                    slice as in_ap so deps attach per AG half)
      s_pc        : [P, C*P] bf16 per core, chunk-major S (dedup-accumulated
                    edge weights, S[row, dst_local])
    """
    src = np.asarray(edge_index[0], dtype=np.int64)
    dst = np.asarray(edge_index[1], dtype=np.int64)

    deg = (np.bincount(dst, minlength=N_NODES) + 1.0).astype(np.float32)
    dinv = (1.0 / np.sqrt(deg.astype(np.float64))).astype(np.float32)

    all_src = np.concatenate([src, np.arange(N_NODES, dtype=np.int64)])
    all_dst = np.concatenate([dst, np.arange(N_NODES, dtype=np.int64)])
    all_w = np.concatenate([dinv[src] * dinv[dst], dinv * dinv]).astype(np.float32)

    per_core = []
    n0 = np.zeros((N_CORES, N_BLOCKS), dtype=np.int64)
    n1 = np.zeros((N_CORES, N_BLOCKS), dtype=np.int64)
    for c in range(N_CORES):
        lo = c * SHARD
        mask = (all_dst >= lo) & (all_dst < lo + SHARD)
        csrc, cdst, cw = all_src[mask], all_dst[mask] - lo, all_w[mask]
        blocks = []
        for b in range(N_BLOCKS):
            bm = (cdst >= b * P) & (cdst < (b + 1) * P)
            bsrc, bdst, bw = csrc[bm], cdst[bm] - b * P, cw[bm]
            uniq, inv = np.unique(bsrc, return_inverse=True)
            in0 = (uniq % SHARD) < SA
            order = np.argsort(~in0, kind="stable")    # set0 rows first
            rank = np.empty(len(uniq), dtype=np.int64)
            rank[order] = np.arange(len(uniq))
            blocks.append((uniq[order], rank[inv], bdst, bw, int(in0.sum())))
            n0[c, b] = in0.sum()
            n1[c, b] = len(uniq) - in0.sum()
        per_core.append(blocks)

    sched0 = [int(x) for x in ((n0.max(axis=0) + P - 1) // P)]
    sched1 = [int(x) for x in ((n1.max(axis=0) + P - 1) // P)]
    stot = [a + b for a, b in zip(sched0, sched1)]
    C = sum(stot)
    cbases = np.concatenate([[0], np.cumsum(stot)])

    gidx1_pc, gidxA_pc, s_pc = [], [], []
    for c in range(N_CORES):
        flat = np.full(C * P, -1, dtype=np.int64)
        is1 = np.zeros(C * P, dtype=bool)
        S = np.zeros((C * P, P), dtype=np.float32)
        for b in range(N_BLOCKS):
            uniq, inv, bdst, bw, u0 = per_core[c][b]
            r0 = cbases[b] * P                     # set0 region
            r1 = r0 + sched0[b] * P                # set1 region
            is1[r1: r1 + sched1[b] * P] = True
            nu = len(uniq)
            pos = np.where(np.arange(nu) < u0,
                           r0 + np.arange(nu), r1 + np.arange(nu) - u0)
            flat[pos] = uniq
            np.add.at(S, (pos[inv], bdst), bw)
        pad = flat < 0
        flat_ids = np.where(pad, 0, flat)
        # Split-AllGather hfull layout:
        # node n = r*SHARD + q -> r*SA + q             (q < SA,  first half)
        #                      -> 8*SA + r*SB + (q-SA) (q >= SA, second half)
        r_, q_ = flat_ids // SHARD, flat_ids % SHARD
        ag = np.where(q_ < SA, r_ * SA + q_, 8 * SA + r_ * SB + (q_ - SA))
        ag = np.where(is1, ag - 8 * SA, ag)        # relative to half-1 slice
        ag = np.where(pad, 0, ag)                  # pads gather slice row 0

        # dma_gather int16 index layout: flat index i -> [i % 16, i // 16],
        # replicated across the 8 GPSIMD-core partition groups.
        def wrap(f):
            w = f.astype(np.int16).reshape(C * P // 16, 16).T
            return np.tile(w, (8, 1)).copy()

        gidx1_pc.append(wrap(flat_ids))
        gidxA_pc.append(wrap(ag))
        S2 = S.reshape(-1, P, P).transpose(1, 0, 2).reshape(P, -1)
        s_pc.append(np.ascontiguousarray(S2).astype(_BF))
    return (tuple(sched0), tuple(sched1)), gidx1_pc, gidxA_pc, s_pc


# ----------------------------------------------------------------------------
# Device kernel
# ----------------------------------------------------------------------------

def _build(schedule, nrep=1):
    sched0, sched1 = schedule
    stot = [a + b for a, b in zip(sched0, sched1)]
    C = sum(stot)
    nc = bacc.Bacc("TRN2", num_devices=N_CORES)

    xg = nc.dram_tensor("xg", [N_NODES, IN_DIM], _DTG, kind="ExternalInput")
    W1 = nc.dram_tensor("W1", [IN_DIM, HID_DIM], _DTG, kind="ExternalInput")
    W2 = nc.dram_tensor("W2", [HID_DIM, HID_DIM], _DTG, kind="ExternalInput")
    W3 = nc.dram_tensor("W3", [HID_DIM, OUT_DIM], _DTG, kind="ExternalInput")
    b1 = nc.dram_tensor("b1", [1, HID_DIM], _DTG, kind="ExternalInput")
    b2 = nc.dram_tensor("b2", [1, HID_DIM], _DTG, kind="ExternalInput")
    b3 = nc.dram_tensor("b3", [1, OUT_DIM], _DTG, kind="ExternalInput")
    gidx1 = nc.dram_tensor("gidx1", [P, C * 8], mybir.dt.int16, kind="ExternalInput")
    gidxA = nc.dram_tensor("gidxA", [P, C * 8], mybir.dt.int16, kind="ExternalInput")
    S = nc.dram_tensor("S", [P, C * P], _DTG, kind="ExternalInput")
    out = nc.dram_tensor("out", [SHARD, OUT_DIM], _DT, kind="ExternalOutput")

    hs1 = nc.dram_tensor("hs1", [SHARD, HID_DIM], _DTG)
    hs3 = nc.dram_tensor("hs3", [SHARD, OUT_DIM], _DTG)
    hf1 = nc.dram_tensor("hf1", [N_NODES, HID_DIM], _DTG, addr_space="Shared")
    hf3 = nc.dram_tensor("hf3", [N_NODES, OUT_DIM], _DTG, addr_space="Shared")

    rg = [list(range(N_CORES))]

    cbases = [0]
    for b in range(N_BLOCKS):
        cbases.append(cbases[-1] + stot[b])

    with tile.TileContext(nc) as tc, ExitStack() as ctx:
        const = ctx.enter_context(tc.tile_pool(name="const", bufs=1))
        gp = ctx.enter_context(tc.tile_pool(name="gather", bufs=12))
        ab = ctx.enter_context(tc.tile_pool(name="aggt", bufs=8))
        hp = ctx.enter_context(tc.tile_pool(name="hb", bufs=3))
        op = ctx.enter_context(tc.tile_pool(name="ob", bufs=8))
        psa = ctx.enter_context(tc.tile_pool(name="psa", bufs=4, space="PSUM"))
        psd = ctx.enter_context(tc.tile_pool(name="psd", bufs=2, space="PSUM"))
        pst = ctx.enter_context(tc.tile_pool(name="pst", bufs=2, space="PSUM"))

        ident = const.tile([P, P], _DTG)
        make_identity(nc, ident[:])
        onesb = const.tile([1, P], _DTG)
        nc.vector.memset(onesb[:], 1.0)

        # gather-critical loads first on the SP queue: gidx1, then S slices
        # (emitted per-block inside the L1 loop)
        gidx1_t = const.tile([P, C * 8], mybir.dt.int16)
        nc.sync.dma_start(out=gidx1_t[:], in_=gidx1[:])
        s_all = const.tile([P, C * P], _DTG)

        # S slices + weights + gidxA go on the Activation DMA queue, ALL
        # emitted up front: none of them has a data dependency, and a
        # dependent DMA (hs writes) parked at a queue head blocks everything
        # behind it on that queue, so dependency-free loads must never queue
        # behind one. Order: S0, W1/b1, S1..S9, W2/b2/W3/b3, gidxA (needed
        # last, for layer 2's gathers).
        s_loads = [
            (s_all[:, cbases[d] * P: cbases[d + 1] * P],
             S[:, cbases[d] * P: cbases[d + 1] * P])
            for d in range(N_BLOCKS)
        ]
        nc.scalar.dma_start(out=s_loads[0][0], in_=s_loads[0][1])

        w_tiles, b_tiles = [], []
        for W, b, fin, fout in [(W1, b1, IN_DIM, HID_DIM),
                                (W2, b2, HID_DIM, HID_DIM),
                                (W3, b3, HID_DIM, OUT_DIM)]:
            nk = fin // P
            wt = const.tile([P, nk * fout], _DTG, tag=f"w{fin}x{fout}")
            for k in range(nk):
                nc.scalar.dma_start(
                    out=wt[:].rearrange("p (k f) -> p k f", k=nk)[:, k:k + 1, :],
                    in_=W[:].rearrange("(k p) f -> p k f", p=P)[:, k:k + 1, :])
            bt = const.tile([1, fout], _DTG, tag=f"b{fout}")
            nc.scalar.dma_start(out=bt[:], in_=b[:])
            w_tiles.append(wt)
            b_tiles.append(bt)
            if fout == HID_DIM and fin == IN_DIM:    # after W1/b1: rest of S
                for dst_ap, src_ap in s_loads[1:]:
                    nc.scalar.dma_start(out=dst_ap, in_=src_ap)

        gidxA_t = const.tile([P, C * 8], mybir.dt.int16)
        nc.scalar.dma_start(out=gidxA_t[:], in_=gidxA[:])

        z2T = const.tile([P, (HID_DIM // P) * SHARD], _DTG)

        def gathers(ranges, gidx_t, fin):
            """Batched gathers over chunk ranges; returns [(ga, gb, gt)].

            ranges: list of (chunk_base, nchunks, in_ap); chunk indices are
            global stream positions. Each AG half is a separate in_ap slice
            so the gather's data dependency attaches to just that half.
            """
            tiles = []
            for cb, n, src in ranges:
                for g0 in range(0, n, GK):
                    g1 = min(g0 + GK, n)
                    n_sub = g1 - g0
                    gt = gp.tile([P, GK * HID_DIM], _DTG, tag="g")
                    nc.gpsimd.dma_gather(
                        out_ap=gt[:, :n_sub * fin].rearrange(
                            "p (c f) -> p c f", c=n_sub),
                        in_ap=src,
                        idxs_ap=gidx_t[:, (cb + g0) * 8: (cb + g1) * 8],
                        num_idxs=n_sub * P,
                        num_idxs_reg=n_sub * P,
                        elem_size=fin,
                    )
                    tiles.append((cb + g0, cb + g1, gt))
            return tiles

        def block_ranges(hf_t, d):
            """(chunk_base, n, in_ap) for dst-block d split by AG half."""
            return [
                (cbases[d], sched0[d], hf_t[:N_CORES * SA, :]),
                (cbases[d] + sched0[d], sched1[d], hf_t[N_CORES * SA:, :]),
            ]

        def half_range(d, half):
            """(first_chunk, nchunks) of dst-block d for AG half / both."""
            if half == 0:
                return cbases[d], sched0[d]
            if half == 1:
                return cbases[d] + sched0[d], sched1[d]
            return cbases[d], stot[d]

        def spmm_aggT(li, d, half=None):
            """Partial/full aggregate of dst-block d, transposed to bf16.

            k is the inner loop; each k accumulates in its OWN full PSUM
            bank (2 KiB zero region), so the interleaved groups can't stomp
            each other and gather tiles free after a single pass.
            half=0/1 aggregates only that AG half's chunks (partials are
            merged later in the dense matmul's accumulation).
            """
            fin = IN_DIM if li == 0 else HID_DIM
            nk = fin // P
            if li == 0:
                first, n = half_range(d, None)
                ranges = [(first, n, xg[:])]
                gidx_t = gidx1_t
            else:
                r0, r1 = block_ranges(hf1, d)
                ranges = [r0, r1] if half is None else [(r0, r1)[half]]
                first = ranges[0][0]
                n = sum(r[1] for r in ranges)
                gidx_t = gidxA_t
            last = first + n - 1
            tiles = gathers(ranges, gidx_t, fin)
            psk = [psa.tile([P, HID_DIM], _DT, tag="psa", name=f"psk{k}")
                   for k in range(nk)]
            for ga, gb, gt in tiles:
                for c in range(ga, gb):
                    for k in range(nk):
                        nc.tensor.matmul(
                            psk[k][:, :P],
                            lhsT=gt[:, (c - ga) * fin + k * P:
                                    (c - ga) * fin + (k + 1) * P],
                            rhs=s_all[:, c * P:(c + 1) * P],
                            start=(c == first),
                            stop=(c == last),
                        )
            at = ab.tile([P, 4 * P], _DTG, tag="at")
            for k in range(nk):
                nc.vector.tensor_copy(at[:, k * P:(k + 1) * P], psk[k][:, :P])
            return at

        def dense(li, d, ats):
            """z_{li+1} block d = tanh(sum_i ats[i]^T @ W + b), node-major."""
            fin = IN_DIM if li == 0 else HID_DIM
            fout = HID_DIM
            nk = fin // P
            nd = min(P, SHARD - d * P)
            wt, bt = w_tiles[li], b_tiles[li]
            ps = psd.tile([P, HID_DIM], _DT, tag="psd")
            for i, at in enumerate(ats):
                for k in range(nk):
                    nc.tensor.matmul(
                        ps[:nd, :fout],
                        lhsT=at[:, k * P:k * P + nd],
                        rhs=wt[:, k * fout:(k + 1) * fout],
                        start=(i == 0 and k == 0),
                        stop=False,
                    )
            nc.tensor.matmul(
                ps[:nd, :fout], lhsT=onesb[:, :nd], rhs=bt[:],
                start=False, stop=True,
            )
            hbt = hp.tile([P, HID_DIM], _DTG, tag="hb")
            nc.scalar.activation(
                hbt[:nd, :fout], ps[:nd, :fout],
                mybir.ActivationFunctionType.Tanh)
            return hbt

        def ag_half(hs_t, hf_t, half):
            if half == 0:
                ins_, outs_ = hs_t[:SA, :], hf_t[:N_CORES * SA, :]
            else:
                ins_, outs_ = hs_t[SA:, :], hf_t[N_CORES * SA:, :]
            nc.gpsimd.collective_compute(
                "AllGather",
                mybir.AluOpType.bypass,
                replica_groups=rg,
                ins=[ins_],
                outs=[outs_],
            )

        # ---- Layer 1: SpMM(x) -> dense W1 -> tanh -> hs1/AG ----
        # Postludes are emitted one block behind the SpMM matmuls so the PE
        # sequencer always has ready chunk-matmul work while a postlude
        # instruction parks on a cross-engine dependency (4-deep wait queue).
        def l1_post(d, at):
            nd = min(P, SHARD - d * P)
            hbt = dense(0, d, [at])
            nc.sync.dma_start(out=hs1[d * P: d * P + nd, :], in_=hbt[:nd, :])
            if d == 7:
                ag_half(hs1, hf1, 0)

        prev = None
        for d in range(N_BLOCKS):
            at = spmm_aggT(0, d)
            if prev is not None:
                l1_post(*prev)
            prev = (d, at)
        l1_post(*prev)
        ag_half(hs1, hf1, 1)

        # ---- Layer 2: SpMM(z1) -> dense W2 -> tanh -> z2T; L3 dense ----
        # Pass A: while AG half-1 is in flight, fully aggregate the first
        # K2 blocks' half-0 chunks into SBUF partials (frees PSUM + gather
        # tiles immediately, keeping the DMA engines fed through the
        # collective's latency).
        K2 = 5
        at0_l2 = [spmm_aggT(1, d, half=0) for d in range(K2)]

        def l2_post(d, ats):
            nd = min(P, SHARD - d * P)
            hbt = dense(1, d, ats)
            for k in range(HID_DIM // P):
                pt = pst.tile([P, P], _DTG, tag="pst")
                nc.tensor.transpose(
                    out=pt[:, :nd],
                    in_=hbt[:nd, k * P:(k + 1) * P],
                    identity=ident[:nd, :nd],
                )
                nc.vector.tensor_copy(
                    z2T[:, k * SHARD + d * P: k * SHARD + d * P + nd],
                    pt[:, :nd],
                )
            ps3 = psd.tile([P, HID_DIM], _DT, tag="psd")
            for k in range(HID_DIM // P):
                nc.tensor.matmul(
                    ps3[:nd, :OUT_DIM],
                    lhsT=z2T[:, k * SHARD + d * P: k * SHARD + d * P + nd],
                    rhs=w_tiles[2][:, k * OUT_DIM:(k + 1) * OUT_DIM],
                    start=(k == 0),
                    stop=(k == HID_DIM // P - 1),
                )
            hb3 = hp.tile([P, HID_DIM], _DTG, tag="hb")
            nc.scalar.activation(
                hb3[:nd, :OUT_DIM], ps3[:nd, :OUT_DIM],
                mybir.ActivationFunctionType.Copy)
            nc.sync.dma_start(
                out=hs3[d * P: d * P + nd, :], in_=hb3[:nd, :OUT_DIM])
            if d == 6:
                ag_half(hs3, hf3, 0)

        prev = None
        for d in range(N_BLOCKS):
            if d < K2:
                ats = [at0_l2[d], spmm_aggT(1, d, half=1)]
            else:
                ats = [spmm_aggT(1, d)]
            if prev is not None:
                l2_post(*prev)
            prev = (d, ats)
        l2_post(*prev)
        ag_half(hs3, hf3, 1)

        # ---- Layer 3: SpMM(h3) + b3 -> out ----
        def spmm3(d, half):
            """One accumulation group of L3's node-major SpMM in PSUM."""
            first, n = half_range(d, half)
            last = first + n - 1
            ranges = block_ranges(hf3, d)
            if half is not None:
                ranges = [ranges[half]]
            ps = psd.tile([P, HID_DIM], _DT, tag="psd")
            for ga, gb, gt in gathers(ranges, gidxA_t, OUT_DIM):
                for c in range(ga, gb):
                    nc.tensor.matmul(
                        ps[:, :OUT_DIM],
                        lhsT=s_all[:, c * P:(c + 1) * P],
                        rhs=gt[:, (c - ga) * OUT_DIM:(c - ga + 1) * OUT_DIM],
                        start=(c == first),
                        stop=(half == 0 and c == last),
                    )
            if half != 0:              # bias closes the group
                nc.tensor.matmul(
                    ps[:, :OUT_DIM], lhsT=onesb[:], rhs=b_tiles[2][:],
                    start=False, stop=True,
                )
            return ps

        # Pass A: half-0 partials for the first K3 blocks (fp32 in SBUF)
        K3 = 6
        ob0_l3 = []
        for d in range(K3):
            ps = spmm3(d, 0)
            ob0 = op.tile([P, OUT_DIM], _DT, tag="ob")
            nc.vector.tensor_copy(ob0[:], ps[:, :OUT_DIM])
            ob0_l3.append(ob0)
        for d in range(N_BLOCKS):
            nd = min(P, SHARD - d * P)
            ps = spmm3(d, 1 if d < K3 else None)
            ob = op.tile([P, OUT_DIM], _DT, tag="ob")
            if d < K3:
                nc.vector.scalar_tensor_tensor(
                    out=ob[:nd], in0=ps[:nd, :OUT_DIM], scalar=1.0,
                    in1=ob0_l3[d][:nd],
                    op0=mybir.AluOpType.mult, op1=mybir.AluOpType.add,
                )
            else:
                nc.vector.tensor_copy(ob[:nd], ps[:nd, :OUT_DIM])
            nc.sync.dma_start(out=out[d * P: d * P + nd, :], in_=ob[:nd])

    nc.compile()
    return nc


_CACHE = {}


def _get_kernel(schedule, nrep=1):
    key = (tuple(schedule), nrep)
    if key not in _CACHE:
        _CACHE[key] = _build(schedule, nrep)
    return _CACHE[key]


# ----------------------------------------------------------------------------
# Entry point
# ----------------------------------------------------------------------------

def kernel(x, W1, b1, W2, b2, W3, b3, edge_index, _trace=False, _trace_kwargs=None):
    x = np.asarray(x, dtype=np.float32)
    Ws = [np.ascontiguousarray(np.asarray(w, dtype=np.float32).astype(_BF))
          for w in (W1, W2, W3)]
    bs = [np.ascontiguousarray(
        np.asarray(b, dtype=np.float32).reshape(1, -1).astype(_BF))
        for b in (b1, b2, b3)]
    edge_index = np.asarray(edge_index)

    xg = np.ascontiguousarray(x.astype(_BF))
    schedule, gidx1_pc, gidxA_pc, s_pc = _preprocess(edge_index)
    nc = _get_kernel(schedule)

    in_maps = []
    for c in range(N_CORES):
        in_maps.append({
            "xg": xg,
            "W1": Ws[0], "W2": Ws[1], "W3": Ws[2],
            "b1": bs[0], "b2": bs[1], "b3": bs[2],
            "gidx1": gidx1_pc[c],
            "gidxA": gidxA_pc[c],
            "S": s_pc[c],
        })

    kwargs = {}
    if _trace:
        kwargs = {"trace": True, "trace_kwargs": _trace_kwargs or {}}
    try:
        res = run_bass_kernel_spmd(
            nc, in_maps, core_ids=list(range(N_CORES)), **kwargs)
    except Exception:
        # transient axon/device errors (e.g. NRT_EXEC_UNIT_UNRECOVERABLE on a
        # cold worker) clear on re-execution; retry once
        res = run_bass_kernel_spmd(
            nc, in_maps, core_ids=list(range(N_CORES)), **kwargs)
    out = np.concatenate([res.results[c]["out"] for c in range(N_CORES)], axis=0)
    if _trace:
        return out, res
    return out


# revision 24
# speedup vs baseline: 1.0150x; 1.0150x over previous
"""3-layer GCN (GCNConv x3, tanh between) on 8 Trainium2 NeuronCores.

Strategy (v2 — "SpMM-first" restructure of the node-range-sharded scheme):
  - GCN aggregation commutes with the dense transform (both linear), so
    layer 1 aggregates the *input features* x directly: each core holds
    the full bf16 copy of x in DRAM as a gather table, so layer-1 message
    gathering starts at t=0 with no AllGather and 256-wide (not 512-wide)
    rows. The aggregated block is then densified locally:
        z1 = tanh(aggT_x^T @ W1 + b1).
  - Layer 2 is also SpMM-first on the AllGathered z1 table. The chunk
    matmuls are emitted transposed (aggT_k = G_k^T @ S, [fin_chunk, dst])
    so the aggregate lands feature-major and feeds the dense matmul's
    stationary operand directly — no transposes for z1/z2-in.
  - Layer 3 is dense-first (h3 = z2 @ W3 is 256-wide, halving both its
    AllGather and its gather traffic); z2 is transposed on the
    TensorEngine (bf16, 1 cyc/row) to feature-major for that matmul.
  - Edges (+ one self-edge per node, weight dinv^2) are bucketed per
    128-dst-node block and *deduplicated by src* within the block
    (~10% fewer gathered rows and chunks); the per-chunk S matrix
    [row, dst_local] accumulates duplicate edge weights. One S table
    serves all three layers. Gathers use the GPSIMD dma_gather extended
    instruction batched 8 chunks / 1024 rows per instruction.
  - All matmuls are bf16 (1 cycle/row on the PE) accumulating fp32 in
    PSUM; biases enter PSUM as a rank-1 ones^T @ b matmul.

Numerics: messages, aggregates, and weights are bf16; accumulation is
fp32. End-to-end relative L2 error vs the fp32 reference is ~5e-3.
Host preprocessing touches only edge_index (sorting/bincount/unique),
the degree-derived edge weights, and bf16 casts of x/W/b.
"""
import sys

if "/opt/trn_rl_repo" not in sys.path:
    sys.path.insert(0, "/opt/trn_rl_repo")

from contextlib import ExitStack

import ml_dtypes
import numpy as np

import concourse.bass as bass
import concourse.bacc as bacc
import concourse.mybir as mybir
import concourse.tile as tile
from concourse.bass_utils import run_bass_kernel_spmd
from concourse.masks import make_identity

P = 128
N_CORES = 8
N_NODES = 10000
SHARD = N_NODES // N_CORES          # 1250
N_BLOCKS = (SHARD + P - 1) // P     # 10 (9 full + one 98-row block)
IN_DIM, HID_DIM, OUT_DIM = 256, 512, 256
SA, SB = 640, SHARD - 640           # split-AllGather half sizes
GK = 8                              # gather chunks per dma_gather instr

_DT = mybir.dt.float32
_DTG = mybir.dt.bfloat16

_BF = ml_dtypes.bfloat16


# ----------------------------------------------------------------------------
# Host-side edge preprocessing
# ----------------------------------------------------------------------------

def _preprocess(edge_index: np.ndarray):
    """Bucket edges by dst block, dedup srcs per block, build S + gidx.

    Within each block the deduped src rows are split by AllGather half
    (set0: q < SA, landing in hf[:8*SA]; set1: q >= SA) so the device can
    gather set0 rows as soon as AG half-0 completes, overlapping half-1's
    collective latency with gather traffic.

    Returns (schedule, gidx1_pc, gidxA_pc, s_pc):
      schedule    : (sched0, sched1) per-block chunk counts (all cores)
      gidx1_pc    : [P, C*8] int16 per core, plain node-id gather indices
                    (for the replicated x table)
      gidxA_pc    : [P, C*8] int16 per core, AG-layout indices; set1 rows
                    are RELATIVE to hf[8*SA:] (gathers use the half-table
                    slice as in_ap so deps attach per AG half)
      s_pc        : [P, C*P] bf16 per core, chunk-major S (dedup-accumulated
                    edge weights, S[row, dst_local])
    """
    src = np.asarray(edge_index[0], dtype=np.int64)
    dst = np.asarray(edge_index[1], dtype=np.int64)

    deg = (np.bincount(dst, minlength=N_NODES) + 1.0).astype(np.float32)
    dinv = (1.0 / np.sqrt(deg.astype(np.float64))).astype(np.float32)

    all_src = np.concatenate([src, np.arange(N_NODES, dtype=np.int64)])
    all_dst = np.concatenate([dst, np.arange(N_NODES, dtype=np.int64)])
    all_w = np.concatenate([dinv[src] * dinv[dst], dinv * dinv]).astype(np.float32)

    per_core = []
    n0 = np.zeros((N_CORES, N_BLOCKS), dtype=np.int64)
    n1 = np.zeros((N_CORES, N_BLOCKS), dtype=np.int64)
    for c in range(N_CORES):
        lo = c * SHARD
        mask = (all_dst >= lo) & (all_dst < lo + SHARD)
        csrc, cdst, cw = all_src[mask], all_dst[mask] - lo, all_w[mask]
        blocks = []
        for b in range(N_BLOCKS):
            bm = (cdst >= b * P) & (cdst < (b + 1) * P)
            bsrc, bdst, bw = csrc[bm], cdst[bm] - b * P, cw[bm]
            uniq, inv = np.unique(bsrc, return_inverse=True)
            in0 = (uniq % SHARD) < SA
            order = np.argsort(~in0, kind="stable")    # set0 rows first
            rank = np.empty(len(uniq), dtype=np.int64)
            rank[order] = np.arange(len(uniq))
            blocks.append((uniq[order], rank[inv], bdst, bw, int(in0.sum())))
            n0[c, b] = in0.sum()
            n1[c, b] = len(uniq) - in0.sum()
        per_core.append(blocks)

    sched0 = [int(x) for x in ((n0.max(axis=0) + P - 1) // P)]
    sched1 = [int(x) for x in ((n1.max(axis=0) + P - 1) // P)]
    stot = [a + b for a, b in zip(sched0, sched1)]
    C = sum(stot)
    cbases = np.concatenate([[0], np.cumsum(stot)])

    gidx1_pc, gidxA_pc, s_pc = [], [], []
    for c in range(N_CORES):
        flat = np.full(C * P, -1, dtype=np.int64)
        is1 = np.zeros(C * P, dtype=bool)
        S = np.zeros((C * P, P), dtype=np.float32)
        for b in range(N_BLOCKS):
            uniq, inv, bdst, bw, u0 = per_core[c][b]
            r0 = cbases[b] * P                     # set0 region
            r1 = r0 + sched0[b] * P                # set1 region
            is1[r1: r1 + sched1[b] * P] = True
            nu = len(uniq)
            pos = np.where(np.arange(nu) < u0,
                           r0 + np.arange(nu), r1 + np.arange(nu) - u0)
            flat[pos] = uniq
            np.add.at(S, (pos[inv], bdst), bw)
        pad = flat < 0
        flat_ids = np.where(pad, 0, flat)
        # Split-AllGather hfull layout:
        # node n = r*SHARD + q -> r*SA + q             (q < SA,  first half)
        #                      -> 8*SA + r*SB + (q-SA) (q >= SA, second half)
        r_, q_ = flat_ids // SHARD, flat_ids % SHARD
        ag = np.where(q_ < SA, r_ * SA + q_, 8 * SA + r_ * SB + (q_ - SA))
        ag = np.where(is1, ag - 8 * SA, ag)        # relative to half-1 slice
        ag = np.where(pad, 0, ag)                  # pads gather slice row 0

        # dma_gather int16 index layout: flat index i -> [i % 16, i // 16],
        # replicated across the 8 GPSIMD-core partition groups.
        def wrap(f):
            w = f.astype(np.int16).reshape(C * P // 16, 16).T
            return np.tile(w, (8, 1)).copy()

        gidx1_pc.append(wrap(flat_ids))
        gidxA_pc.append(wrap(ag))
        S2 = S.reshape(-1, P, P).transpose(1, 0, 2).reshape(P, -1)
        s_pc.append(np.ascontiguousarray(S2).astype(_BF))
    return (tuple(sched0), tuple(sched1)), gidx1_pc, gidxA_pc, s_pc


# ----------------------------------------------------------------------------
# Device kernel
# ----------------------------------------------------------------------------

def _build(schedule, nrep=1):
    sched0, sched1 = schedule
    stot = [a + b for a, b in zip(sched0, sched1)]
    C = sum(stot)
    nc = bacc.Bacc("TRN2", num_devices=N_CORES)

    xg = nc.dram_tensor("xg", [N_NODES, IN_DIM], _DTG, kind="ExternalInput")
    W1 = nc.dram_tensor("W1", [IN_DIM, HID_DIM], _DTG, kind="ExternalInput")
    W2 = nc.dram_tensor("W2", [HID_DIM, HID_DIM], _DTG, kind="ExternalInput")
    W3 = nc.dram_tensor("W3", [HID_DIM, OUT_DIM], _DTG, kind="ExternalInput")
    b1 = nc.dram_tensor("b1", [1, HID_DIM], _DTG, kind="ExternalInput")
    b2 = nc.dram_tensor("b2", [1, HID_DIM], _DTG, kind="ExternalInput")
    b3 = nc.dram_tensor("b3", [1, OUT_DIM], _DTG, kind="ExternalInput")
    gidx1 = nc.dram_tensor("gidx1", [P, C * 8], mybir.dt.int16, kind="ExternalInput")
    gidxA = nc.dram_tensor("gidxA", [P, C * 8], mybir.dt.int16, kind="ExternalInput")
    S = nc.dram_tensor("S", [P, C * P], _DTG, kind="ExternalInput")
    out = nc.dram_tensor("out", [SHARD, OUT_DIM], _DT, kind="ExternalOutput")

    hs1 = nc.dram_tensor("hs1", [SHARD, HID_DIM], _DTG)
    hs3 = nc.dram_tensor("hs3", [SHARD, OUT_DIM], _DTG)
    hf1 = nc.dram_tensor("hf1", [N_NODES, HID_DIM], _DTG, addr_space="Shared")
    hf3 = nc.dram_tensor("hf3", [N_NODES, OUT_DIM], _DTG, addr_space="Shared")

    rg = [list(range(N_CORES))]

    cbases = [0]
    for b in range(N_BLOCKS):
        cbases.append(cbases[-1] + stot[b])

    with tile.TileContext(nc) as tc, ExitStack() as ctx:
        const = ctx.enter_context(tc.tile_pool(name="const", bufs=1))
        gp = ctx.enter_context(tc.tile_pool(name="gather", bufs=12))
        ab = ctx.enter_context(tc.tile_pool(name="aggt", bufs=8))
        hp = ctx.enter_context(tc.tile_pool(name="hb", bufs=3))
        op = ctx.enter_context(tc.tile_pool(name="ob", bufs=8))
        psa = ctx.enter_context(tc.tile_pool(name="psa", bufs=4, space="PSUM"))
        psd = ctx.enter_context(tc.tile_pool(name="psd", bufs=2, space="PSUM"))
        pst = ctx.enter_context(tc.tile_pool(name="pst", bufs=2, space="PSUM"))

        ident = const.tile([P, P], _DTG)
        make_identity(nc, ident[:])
        onesb = const.tile([1, P], _DTG)
        nc.vector.memset(onesb[:], 1.0)

        # gather-critical loads first on the SP queue: gidx1, then S slices
        # (emitted per-block inside the L1 loop)
        gidx1_t = const.tile([P, C * 8], mybir.dt.int16)
        nc.sync.dma_start(out=gidx1_t[:], in_=gidx1[:])
        s_all = const.tile([P, C * P], _DTG)

        # S slices + weights + gidxA go on the Activation DMA queue, ALL
        # emitted up front: none of them has a data dependency, and a
        # dependent DMA (hs writes) parked at a queue head blocks everything
        # behind it on that queue, so dependency-free loads must never queue
        # behind one. Order: S0, W1/b1, S1..S9, W2/b2/W3/b3, gidxA (needed
        # last, for layer 2's gathers).
        s_loads = [
            (s_all[:, cbases[d] * P: cbases[d + 1] * P],
             S[:, cbases[d] * P: cbases[d + 1] * P])
            for d in range(N_BLOCKS)
        ]
        nc.scalar.dma_start(out=s_loads[0][0], in_=s_loads[0][1])

        w_tiles, b_tiles = [], []
        for W, b, fin, fout in [(W1, b1, IN_DIM, HID_DIM),
                                (W2, b2, HID_DIM, HID_DIM),
                                (W3, b3, HID_DIM, OUT_DIM)]:
            nk = fin // P
            wt = const.tile([P, nk * fout], _DTG, tag=f"w{fin}x{fout}")
            for k in range(nk):
                nc.scalar.dma_start(
                    out=wt[:].rearrange("p (k f) -> p k f", k=nk)[:, k:k + 1, :],
                    in_=W[:].rearrange("(k p) f -> p k f", p=P)[:, k:k + 1, :])
            bt = const.tile([1, fout], _DTG, tag=f"b{fout}")
            nc.scalar.dma_start(out=bt[:], in_=b[:])
            w_tiles.append(wt)
            b_tiles.append(bt)
            if fout == HID_DIM and fin == IN_DIM:    # after W1/b1: rest of S
                for dst_ap, src_ap in s_loads[1:]:
                    nc.scalar.dma_start(out=dst_ap, in_=src_ap)

        gidxA_t = const.tile([P, C * 8], mybir.dt.int16)
        nc.scalar.dma_start(out=gidxA_t[:], in_=gidxA[:])

        z2T = const.tile([P, (HID_DIM // P) * SHARD], _DTG)

        def gathers(ranges, gidx_t, fin):
            """Batched gathers over chunk ranges; returns [(ga, gb, gt)].

            ranges: list of (chunk_base, nchunks, in_ap); chunk indices are
            global stream positions. Each AG half is a separate in_ap slice
            so the gather's data dependency attaches to just that half.
            """
            tiles = []
            for cb, n, src in ranges:
                for g0 in range(0, n, GK):
                    g1 = min(g0 + GK, n)
                    n_sub = g1 - g0
                    gt = gp.tile([P, GK * HID_DIM], _DTG, tag="g")
                    nc.gpsimd.dma_gather(
                        out_ap=gt[:, :n_sub * fin].rearrange(
                            "p (c f) -> p c f", c=n_sub),
                        in_ap=src,
                        idxs_ap=gidx_t[:, (cb + g0) * 8: (cb + g1) * 8],
                        num_idxs=n_sub * P,
                        num_idxs_reg=n_sub * P,
                        elem_size=fin,
                    )
                    tiles.append((cb + g0, cb + g1, gt))
            return tiles

        def block_ranges(hf_t, d):
            """(chunk_base, n, in_ap) for dst-block d split by AG half."""
            return [
                (cbases[d], sched0[d], hf_t[:N_CORES * SA, :]),
                (cbases[d] + sched0[d], sched1[d], hf_t[N_CORES * SA:, :]),
            ]

        def half_range(d, half):
            """(first_chunk, nchunks) of dst-block d for AG half / both."""
            if half == 0:
                return cbases[d], sched0[d]
            if half == 1:
                return cbases[d] + sched0[d], sched1[d]
            return cbases[d], stot[d]

        def spmm_aggT(li, d, half=None):
            """Partial/full aggregate of dst-block d, transposed to bf16.

            k is the inner loop; each k accumulates in its OWN full PSUM
            bank (2 KiB zero region), so the interleaved groups can't stomp
            each other and gather tiles free after a single pass.
            half=0/1 aggregates only that AG half's chunks (partials are
            merged later in the dense matmul's accumulation).
            """
            fin = IN_DIM if li == 0 else HID_DIM
            nk = fin // P
            if li == 0:
                first, n = half_range(d, None)
                ranges = [(first, n, xg[:])]
                gidx_t = gidx1_t
            else:
                r0, r1 = block_ranges(hf1, d)
                ranges = [r0, r1] if half is None else [(r0, r1)[half]]
                first = ranges[0][0]
                n = sum(r[1] for r in ranges)
                gidx_t = gidxA_t
            last = first + n - 1
            tiles = gathers(ranges, gidx_t, fin)
            psk = [psa.tile([P, HID_DIM], _DT, tag="psa", name=f"psk{k}")
                   for k in range(nk)]
            for ga, gb, gt in tiles:
                for c in range(ga, gb):
                    for k in range(nk):
                        nc.tensor.matmul(
                            psk[k][:, :P],
                            lhsT=gt[:, (c - ga) * fin + k * P:
                                    (c - ga) * fin + (k + 1) * P],
                            rhs=s_all[:, c * P:(c + 1) * P],
                            start=(c == first),
                            stop=(c == last),
                        )
            at = ab.tile([P, 4 * P], _DTG, tag="at")
            for k in range(nk):
                nc.vector.tensor_copy(at[:, k * P:(k + 1) * P], psk[k][:, :P])
            return at

        def dense(li, d, ats):
            """z_{li+1} block d = tanh(sum_i ats[i]^T @ W + b), node-major."""
            fin = IN_DIM if li == 0 else HID_DIM
            fout = HID_DIM
            nk = fin // P
            nd = min(P, SHARD - d * P)
            wt, bt = w_tiles[li], b_tiles[li]
            ps = psd.tile([P, HID_DIM], _DT, tag="psd")
            for i, at in enumerate(ats):
                for k in range(nk):
                    nc.tensor.matmul(
                        ps[:nd, :fout],
                        lhsT=at[:, k * P:k * P + nd],
                        rhs=wt[:, k * fout:(k + 1) * fout],
                        start=(i == 0 and k == 0),
                        stop=False,
                    )
            nc.tensor.matmul(
                ps[:nd, :fout], lhsT=onesb[:, :nd], rhs=bt[:],
                start=False, stop=True,
            )
            hbt = hp.tile([P, HID_DIM], _DTG, tag="hb")
            nc.scalar.activation(
                hbt[:nd, :fout], ps[:nd, :fout],
                mybir.ActivationFunctionType.Tanh)
            return hbt

        def ag_half(hs_t, hf_t, half):
            if half == 0:
                ins_, outs_ = hs_t[:SA, :], hf_t[:N_CORES * SA, :]
            else:
                ins_, outs_ = hs_t[SA:, :], hf_t[N_CORES * SA:, :]
            nc.gpsimd.collective_compute(
                "AllGather",
                mybir.AluOpType.bypass,
                replica_groups=rg,
                ins=[ins_],
                outs=[outs_],
            )

        # ---- Layer 1: SpMM(x) -> dense W1 -> tanh -> hs1/AG ----
        # Postludes are emitted one block behind the SpMM matmuls so the PE
        # sequencer always has ready chunk-matmul work while a postlude
        # instruction parks on a cross-engine dependency (4-deep wait queue).
        def l1_post(d, at):
            nd = min(P, SHARD - d * P)
            hbt = dense(0, d, [at])
            nc.sync.dma_start(out=hs1[d * P: d * P + nd, :], in_=hbt[:nd, :])
            if d == 4:
                ag_half(hs1, hf1, 0)

        prev = None
        for d in range(N_BLOCKS):
            at = spmm_aggT(0, d)
            if prev is not None:
                l1_post(*prev)
            prev = (d, at)
        l1_post(*prev)
        ag_half(hs1, hf1, 1)

        # ---- Layer 2: SpMM(z1) -> dense W2 -> tanh -> z2T; L3 dense ----
        # Pass A: while AG half-1 is in flight, fully aggregate the first
        # K2 blocks' half-0 chunks into SBUF partials (frees PSUM + gather
        # tiles immediately, keeping the DMA engines fed through the
        # collective's latency).
        K2 = 5
        at0_l2 = [spmm_aggT(1, d, half=0) for d in range(K2)]

        def l2_post(d, ats):
            nd = min(P, SHARD - d * P)
            hbt = dense(1, d, ats)
            for k in range(HID_DIM // P):
                pt = pst.tile([P, P], _DTG, tag="pst")
                nc.tensor.transpose(
                    out=pt[:, :nd],
                    in_=hbt[:nd, k * P:(k + 1) * P],
                    identity=ident[:nd, :nd],
                )
                nc.vector.tensor_copy(
                    z2T[:, k * SHARD + d * P: k * SHARD + d * P + nd],
                    pt[:, :nd],
                )
            ps3 = psd.tile([P, HID_DIM], _DT, tag="psd")
            for k in range(HID_DIM // P):
                nc.tensor.matmul(
                    ps3[:nd, :OUT_DIM],
                    lhsT=z2T[:, k * SHARD + d * P: k * SHARD + d * P + nd],
                    rhs=w_tiles[2][:, k * OUT_DIM:(k + 1) * OUT_DIM],
                    start=(k == 0),
                    stop=(k == HID_DIM // P - 1),
                )
            hb3 = hp.tile([P, HID_DIM], _DTG, tag="hb")
            nc.scalar.activation(
                hb3[:nd, :OUT_DIM], ps3[:nd, :OUT_DIM],
                mybir.ActivationFunctionType.Copy)
            nc.sync.dma_start(
                out=hs3[d * P: d * P + nd, :], in_=hb3[:nd, :OUT_DIM])
            if d == 4:
                ag_half(hs3, hf3, 0)

        prev = None
        for d in range(N_BLOCKS):
            if d < K2:
                ats = [at0_l2[d], spmm_aggT(1, d, half=1)]
            else:
                ats = [spmm_aggT(1, d)]
            if prev is not None:
                l2_post(*prev)
            prev = (d, ats)
        l2_post(*prev)
        ag_half(hs3, hf3, 1)

        # ---- Layer 3: SpMM(h3) + b3 -> out ----
        def spmm3(d, half):
            """One accumulation group of L3's node-major SpMM in PSUM."""
            first, n = half_range(d, half)
            last = first + n - 1
            ranges = block_ranges(hf3, d)
            if half is not None:
                ranges = [ranges[half]]
            ps = psd.tile([P, HID_DIM], _DT, tag="psd")
            for ga, gb, gt in gathers(ranges, gidxA_t, OUT_DIM):
                for c in range(ga, gb):
                    nc.tensor.matmul(
                        ps[:, :OUT_DIM],
                        lhsT=s_all[:, c * P:(c + 1) * P],
                        rhs=gt[:, (c - ga) * OUT_DIM:(c - ga + 1) * OUT_DIM],
                        start=(c == first),
                        stop=(half == 0 and c == last),
                    )
            if half != 0:              # bias closes the group
                nc.tensor.matmul(
                    ps[:, :OUT_DIM], lhsT=onesb[:], rhs=b_tiles[2][:],
                    start=False, stop=True,
                )
            return ps

        # Pass A: half-0 partials for the first K3 blocks (fp32 in SBUF)
        K3 = 6
        ob0_l3 = []
        for d in range(K3):
            ps = spmm3(d, 0)
            ob0 = op.tile([P, OUT_DIM], _DT, tag="ob")
            nc.vector.tensor_copy(ob0[:], ps[:, :OUT_DIM])
            ob0_l3.append(ob0)
        for d in range(N_BLOCKS):
            nd = min(P, SHARD - d * P)
            ps = spmm3(d, 1 if d < K3 else None)
            ob = op.tile([P, OUT_DIM], _DT, tag="ob")
            if d < K3:
                nc.vector.scalar_tensor_tensor(
                    out=ob[:nd], in0=ps[:nd, :OUT_DIM], scalar=1.0,
                    in1=ob0_l3[d][:nd],
                    op0=mybir.AluOpType.mult, op1=mybir.AluOpType.add,
                )
            else:
                nc.vector.tensor_copy(ob[:nd], ps[:nd, :OUT_DIM])
            nc.sync.dma_start(out=out[d * P: d * P + nd, :], in_=ob[:nd])

    nc.compile()
    return nc


_CACHE = {}


def _get_kernel(schedule, nrep=1):
    key = (tuple(schedule), nrep)
    if key not in _CACHE:
        _CACHE[key] = _build(schedule, nrep)
    return _CACHE[key]


# ----------------------------------------------------------------------------
# Entry point
# ----------------------------------------------------------------------------

def kernel(x, W1, b1, W2, b2, W3, b3, edge_index, _trace=False, _trace_kwargs=None):
    x = np.asarray(x, dtype=np.float32)
    Ws = [np.ascontiguousarray(np.asarray(w, dtype=np.float32).astype(_BF))
          for w in (W1, W2, W3)]
    bs = [np.ascontiguousarray(
        np.asarray(b, dtype=np.float32).reshape(1, -1).astype(_BF))
        for b in (b1, b2, b3)]
    edge_index = np.asarray(edge_index)

    xg = np.ascontiguousarray(x.astype(_BF))
    schedule, gidx1_pc, gidxA_pc, s_pc = _preprocess(edge_index)
    nc = _get_kernel(schedule)

    in_maps = []
    for c in range(N_CORES):
        in_maps.append({
            "xg": xg,
            "W1": Ws[0], "W2": Ws[1], "W3": Ws[2],
            "b1": bs[0], "b2": bs[1], "b3": bs[2],
            "gidx1": gidx1_pc[c],
            "gidxA": gidxA_pc[c],
            "S": s_pc[c],
        })

    kwargs = {}
    if _trace:
        kwargs = {"trace": True, "trace_kwargs": _trace_kwargs or {}}
    try:
        res = run_bass_kernel_spmd(
            nc, in_maps, core_ids=list(range(N_CORES)), **kwargs)
    except Exception:
        # transient axon/device errors (e.g. NRT_EXEC_UNIT_UNRECOVERABLE on a
        # cold worker) clear on re-execution; retry once
        res = run_bass_kernel_spmd(
            nc, in_maps, core_ids=list(range(N_CORES)), **kwargs)
    out = np.concatenate([res.results[c]["out"] for c in range(N_CORES)], axis=0)
    if _trace:
        return out, res
    return out


# revision 25
# speedup vs baseline: 1.0901x; 1.0739x over previous
"""3-layer GCN (GCNConv x3, tanh between) on 8 Trainium2 NeuronCores.

Strategy (v2 — "SpMM-first" restructure of the node-range-sharded scheme):
  - GCN aggregation commutes with the dense transform (both linear), so
    layer 1 aggregates the *input features* x directly: each core holds
    the full bf16 copy of x in DRAM as a gather table, so layer-1 message
    gathering starts at t=0 with no AllGather and 256-wide (not 512-wide)
    rows. The aggregated block is then densified locally:
        z1 = tanh(aggT_x^T @ W1 + b1).
  - Layer 2 is also SpMM-first on the AllGathered z1 table. The chunk
    matmuls are emitted transposed (aggT_k = G_k^T @ S, [fin_chunk, dst])
    so the aggregate lands feature-major and feeds the dense matmul's
    stationary operand directly — no transposes for z1/z2-in.
  - Layer 3 is dense-first (h3 = z2 @ W3 is 256-wide, halving both its
    AllGather and its gather traffic); z2 is transposed on the
    TensorEngine (bf16, 1 cyc/row) to feature-major for that matmul.
  - Edges (+ one self-edge per node, weight dinv^2) are bucketed per
    128-dst-node block and *deduplicated by src* within the block
    (~10% fewer gathered rows and chunks); the per-chunk S matrix
    [row, dst_local] accumulates duplicate edge weights. One S table
    serves all three layers. Gathers use the GPSIMD dma_gather extended
    instruction batched 8 chunks / 1024 rows per instruction.
  - All matmuls are bf16 (1 cycle/row on the PE) accumulating fp32 in
    PSUM; biases enter PSUM as a rank-1 ones^T @ b matmul.

Numerics: messages, aggregates, and weights are bf16; accumulation is
fp32. End-to-end relative L2 error vs the fp32 reference is ~5e-3.
Host preprocessing touches only edge_index (sorting/bincount/unique),
the degree-derived edge weights, and bf16 casts of x/W/b.
"""
import sys

if "/opt/trn_rl_repo" not in sys.path:
    sys.path.insert(0, "/opt/trn_rl_repo")

from contextlib import ExitStack

import ml_dtypes
import numpy as np

import concourse.bass as bass
import concourse.bacc as bacc
import concourse.mybir as mybir
import concourse.tile as tile
from concourse.bass_utils import run_bass_kernel_spmd
from concourse.masks import make_identity

P = 128
N_CORES = 8
N_NODES = 10000
SHARD = N_NODES // N_CORES          # 1250
N_BLOCKS = (SHARD + P - 1) // P     # 10 (9 full + one 98-row block)
IN_DIM, HID_DIM, OUT_DIM = 256, 512, 256
SA, SB = 768, SHARD - 768           # split-AllGather half sizes
GK = 8                              # gather chunks per dma_gather instr

_DT = mybir.dt.float32
_DTG = mybir.dt.bfloat16

_BF = ml_dtypes.bfloat16


# ----------------------------------------------------------------------------
# Host-side edge preprocessing
# ----------------------------------------------------------------------------

def _preprocess(edge_index: np.ndarray):
    """Bucket edges by dst block, dedup srcs per block, build S + gidx.

    Within each block the deduped src rows are split by AllGather half
    (set0: q < SA, landing in hf[:8*SA]; set1: q >= SA) so the device can
    gather set0 rows as soon as AG half-0 completes, overlapping half-1's
    collective latency with gather traffic.

    Returns (schedule, gidx1_pc, gidxA_pc, s_pc):
      schedule    : (sched0, sched1) per-block chunk counts (all cores)
      gidx1_pc    : [P, C*8] int16 per core, plain node-id gather indices
                    (for the replicated x table)
      gidxA_pc    : [P, C*8] int16 per core, AG-layout indices; set1 rows
                    are RELATIVE to hf[8*SA:] (gathers use the half-table
                    slice as in_ap so deps attach per AG half)
      s_pc        : [P, C*P] bf16 per core, chunk-major S (dedup-accumulated
                    edge weights, S[row, dst_local])
    """
    src = np.asarray(edge_index[0], dtype=np.int64)
    dst = np.asarray(edge_index[1], dtype=np.int64)

    deg = (np.bincount(dst, minlength=N_NODES) + 1.0).astype(np.float32)
    dinv = (1.0 / np.sqrt(deg.astype(np.float64))).astype(np.float32)

    all_src = np.concatenate([src, np.arange(N_NODES, dtype=np.int64)])
    all_dst = np.concatenate([dst, np.arange(N_NODES, dtype=np.int64)])
    all_w = np.concatenate([dinv[src] * dinv[dst], dinv * dinv]).astype(np.float32)

    per_core = []
    n0 = np.zeros((N_CORES, N_BLOCKS), dtype=np.int64)
    n1 = np.zeros((N_CORES, N_BLOCKS), dtype=np.int64)
    for c in range(N_CORES):
        lo = c * SHARD
        mask = (all_dst >= lo) & (all_dst < lo + SHARD)
        csrc, cdst, cw = all_src[mask], all_dst[mask] - lo, all_w[mask]
        blocks = []
        for b in range(N_BLOCKS):
            bm = (cdst >= b * P) & (cdst < (b + 1) * P)
            bsrc, bdst, bw = csrc[bm], cdst[bm] - b * P, cw[bm]
            uniq, inv = np.unique(bsrc, return_inverse=True)
            in0 = (uniq % SHARD) < SA
            order = np.argsort(~in0, kind="stable")    # set0 rows first
            rank = np.empty(len(uniq), dtype=np.int64)
            rank[order] = np.arange(len(uniq))
            blocks.append((uniq[order], rank[inv], bdst, bw, int(in0.sum())))
            n0[c, b] = in0.sum()
            n1[c, b] = len(uniq) - in0.sum()
        per_core.append(blocks)

    sched0 = [int(x) for x in ((n0.max(axis=0) + P - 1) // P)]
    sched1 = [int(x) for x in ((n1.max(axis=0) + P - 1) // P)]
    stot = [a + b for a, b in zip(sched0, sched1)]
    C = sum(stot)
    cbases = np.concatenate([[0], np.cumsum(stot)])

    gidx1_pc, gidxA_pc, s_pc = [], [], []
    for c in range(N_CORES):
        flat = np.full(C * P, -1, dtype=np.int64)
        is1 = np.zeros(C * P, dtype=bool)
        S = np.zeros((C * P, P), dtype=np.float32)
        for b in range(N_BLOCKS):
            uniq, inv, bdst, bw, u0 = per_core[c][b]
            r0 = cbases[b] * P                     # set0 region
            r1 = r0 + sched0[b] * P                # set1 region
            is1[r1: r1 + sched1[b] * P] = True
            nu = len(uniq)
            pos = np.where(np.arange(nu) < u0,
                           r0 + np.arange(nu), r1 + np.arange(nu) - u0)
            flat[pos] = uniq
            np.add.at(S, (pos[inv], bdst), bw)
        pad = flat < 0
        flat_ids = np.where(pad, 0, flat)
        # Split-AllGather hfull layout:
        # node n = r*SHARD + q -> r*SA + q             (q < SA,  first half)
        #                      -> 8*SA + r*SB + (q-SA) (q >= SA, second half)
        r_, q_ = flat_ids // SHARD, flat_ids % SHARD
        ag = np.where(q_ < SA, r_ * SA + q_, 8 * SA + r_ * SB + (q_ - SA))
        ag = np.where(is1, ag - 8 * SA, ag)        # relative to half-1 slice
        ag = np.where(pad, 0, ag)                  # pads gather slice row 0

        # dma_gather int16 index layout: flat index i -> [i % 16, i // 16],
        # replicated across the 8 GPSIMD-core partition groups.
        def wrap(f):
            w = f.astype(np.int16).reshape(C * P // 16, 16).T
            return np.tile(w, (8, 1)).copy()

        gidx1_pc.append(wrap(flat_ids))
        gidxA_pc.append(wrap(ag))
        S2 = S.reshape(-1, P, P).transpose(1, 0, 2).reshape(P, -1)
        s_pc.append(np.ascontiguousarray(S2).astype(_BF))
    return (tuple(sched0), tuple(sched1)), gidx1_pc, gidxA_pc, s_pc


# ----------------------------------------------------------------------------
# Device kernel
# ----------------------------------------------------------------------------

def _build(schedule, nrep=1):
    sched0, sched1 = schedule
    stot = [a + b for a, b in zip(sched0, sched1)]
    C = sum(stot)
    nc = bacc.Bacc("TRN2", num_devices=N_CORES)

    xg = nc.dram_tensor("xg", [N_NODES, IN_DIM], _DTG, kind="ExternalInput")
    W1 = nc.dram_tensor("W1", [IN_DIM, HID_DIM], _DTG, kind="ExternalInput")
    W2 = nc.dram_tensor("W2", [HID_DIM, HID_DIM], _DTG, kind="ExternalInput")
    W3 = nc.dram_tensor("W3", [HID_DIM, OUT_DIM], _DTG, kind="ExternalInput")
    b1 = nc.dram_tensor("b1", [1, HID_DIM], _DTG, kind="ExternalInput")
    b2 = nc.dram_tensor("b2", [1, HID_DIM], _DTG, kind="ExternalInput")
    b3 = nc.dram_tensor("b3", [1, OUT_DIM], _DTG, kind="ExternalInput")
    gidx1 = nc.dram_tensor("gidx1", [P, C * 8], mybir.dt.int16, kind="ExternalInput")
    gidxA = nc.dram_tensor("gidxA", [P, C * 8], mybir.dt.int16, kind="ExternalInput")
    S = nc.dram_tensor("S", [P, C * P], _DTG, kind="ExternalInput")
    out = nc.dram_tensor("out", [SHARD, OUT_DIM], _DT, kind="ExternalOutput")

    hs1 = nc.dram_tensor("hs1", [SHARD, HID_DIM], _DTG)
    hs3 = nc.dram_tensor("hs3", [SHARD, OUT_DIM], _DTG)
    hf1 = nc.dram_tensor("hf1", [N_NODES, HID_DIM], _DTG, addr_space="Shared")
    hf3 = nc.dram_tensor("hf3", [N_NODES, OUT_DIM], _DTG, addr_space="Shared")

    rg = [list(range(N_CORES))]

    cbases = [0]
    for b in range(N_BLOCKS):
        cbases.append(cbases[-1] + stot[b])

    with tile.TileContext(nc) as tc, ExitStack() as ctx:
        const = ctx.enter_context(tc.tile_pool(name="const", bufs=1))
        gp = ctx.enter_context(tc.tile_pool(name="gather", bufs=6))
        ab = ctx.enter_context(tc.tile_pool(name="aggt", bufs=8))
        hp = ctx.enter_context(tc.tile_pool(name="hb", bufs=3))
        op = ctx.enter_context(tc.tile_pool(name="ob", bufs=8))
        psa = ctx.enter_context(tc.tile_pool(name="psa", bufs=4, space="PSUM"))
        psd = ctx.enter_context(tc.tile_pool(name="psd", bufs=2, space="PSUM"))
        pst = ctx.enter_context(tc.tile_pool(name="pst", bufs=2, space="PSUM"))

        ident = const.tile([P, P], _DTG)
        make_identity(nc, ident[:])
        onesb = const.tile([1, P], _DTG)
        nc.vector.memset(onesb[:], 1.0)

        # gather-critical loads first on the SP queue: gidx1, then S slices
        # (emitted per-block inside the L1 loop)
        gidx1_t = const.tile([P, C * 8], mybir.dt.int16)
        nc.sync.dma_start(out=gidx1_t[:], in_=gidx1[:])
        s_all = const.tile([P, C * P], _DTG)

        # S slices + weights + gidxA go on the Activation DMA queue, ALL
        # emitted up front: none of them has a data dependency, and a
        # dependent DMA (hs writes) parked at a queue head blocks everything
        # behind it on that queue, so dependency-free loads must never queue
        # behind one. Order: S0, W1/b1, S1..S9, W2/b2/W3/b3, gidxA (needed
        # last, for layer 2's gathers).
        s_loads = [
            (s_all[:, cbases[d] * P: cbases[d + 1] * P],
             S[:, cbases[d] * P: cbases[d + 1] * P])
            for d in range(N_BLOCKS)
        ]
        nc.scalar.dma_start(out=s_loads[0][0], in_=s_loads[0][1])

        w_tiles, b_tiles = [], []
        for W, b, fin, fout in [(W1, b1, IN_DIM, HID_DIM),
                                (W2, b2, HID_DIM, HID_DIM),
                                (W3, b3, HID_DIM, OUT_DIM)]:
            nk = fin // P
            wt = const.tile([P, nk * fout], _DTG, tag=f"w{fin}x{fout}")
            for k in range(nk):
                nc.scalar.dma_start(
                    out=wt[:].rearrange("p (k f) -> p k f", k=nk)[:, k:k + 1, :],
                    in_=W[:].rearrange("(k p) f -> p k f", p=P)[:, k:k + 1, :])
            bt = const.tile([1, fout], _DTG, tag=f"b{fout}")
            nc.scalar.dma_start(out=bt[:], in_=b[:])
            w_tiles.append(wt)
            b_tiles.append(bt)
            if fout == HID_DIM and fin == IN_DIM:    # after W1/b1: rest of S
                for dst_ap, src_ap in s_loads[1:]:
                    nc.scalar.dma_start(out=dst_ap, in_=src_ap)

        gidxA_t = const.tile([P, C * 8], mybir.dt.int16)
        nc.scalar.dma_start(out=gidxA_t[:], in_=gidxA[:])

        z2T = const.tile([P, (HID_DIM // P) * SHARD], _DTG)

        def gathers(ranges, gidx_t, fin):
            """Batched gathers over chunk ranges; returns [(ga, gb, gt)].

            ranges: list of (chunk_base, nchunks, in_ap); chunk indices are
            global stream positions. Each AG half is a separate in_ap slice
            so the gather's data dependency attaches to just that half.
            """
            tiles = []
            for cb, n, src in ranges:
                for g0 in range(0, n, GK):
                    g1 = min(g0 + GK, n)
                    n_sub = g1 - g0
                    gt = gp.tile([P, GK * HID_DIM], _DTG, tag="g")
                    nc.gpsimd.dma_gather(
                        out_ap=gt[:, :n_sub * fin].rearrange(
                            "p (c f) -> p c f", c=n_sub),
                        in_ap=src,
                        idxs_ap=gidx_t[:, (cb + g0) * 8: (cb + g1) * 8],
                        num_idxs=n_sub * P,
                        num_idxs_reg=n_sub * P,
                        elem_size=fin,
                    )
                    tiles.append((cb + g0, cb + g1, gt))
            return tiles

        def block_ranges(hf_t, d):
            """(chunk_base, n, in_ap) for dst-block d split by AG half."""
            return [
                (cbases[d], sched0[d], hf_t[:N_CORES * SA, :]),
                (cbases[d] + sched0[d], sched1[d], hf_t[N_CORES * SA:, :]),
            ]

        def half_range(d, half):
            """(first_chunk, nchunks) of dst-block d for AG half / both."""
            if half == 0:
                return cbases[d], sched0[d]
            if half == 1:
                return cbases[d] + sched0[d], sched1[d]
            return cbases[d], stot[d]

        def spmm_aggT(li, d, half=None):
            """Partial/full aggregate of dst-block d, transposed to bf16.

            k is the inner loop; each k accumulates in its OWN full PSUM
            bank (2 KiB zero region), so the interleaved groups can't stomp
            each other and gather tiles free after a single pass.
            half=0/1 aggregates only that AG half's chunks (partials are
            merged later in the dense matmul's accumulation).
            """
            fin = IN_DIM if li == 0 else HID_DIM
            nk = fin // P
            if li == 0:
                first, n = half_range(d, None)
                ranges = [(first, n, xg[:])]
                gidx_t = gidx1_t
            else:
                r0, r1 = block_ranges(hf1, d)
                ranges = [r0, r1] if half is None else [(r0, r1)[half]]
                first = ranges[0][0]
                n = sum(r[1] for r in ranges)
                gidx_t = gidxA_t
            last = first + n - 1
            tiles = gathers(ranges, gidx_t, fin)
            psk = [psa.tile([P, HID_DIM], _DT, tag="psa", name=f"psk{k}")
                   for k in range(nk)]
            for ga, gb, gt in tiles:
                for c in range(ga, gb):
                    for k in range(nk):
                        nc.tensor.matmul(
                            psk[k][:, :P],
                            lhsT=gt[:, (c - ga) * fin + k * P:
                                    (c - ga) * fin + (k + 1) * P],
                            rhs=s_all[:, c * P:(c + 1) * P],
                            start=(c == first),
                            stop=(c == last),
                        )
            at = ab.tile([P, 4 * P], _DTG, tag="at")
            for k in range(nk):
                nc.vector.tensor_copy(at[:, k * P:(k + 1) * P], psk[k][:, :P])
            return at

        def dense(li, d, ats):
            """z_{li+1} block d = tanh(sum_i ats[i]^T @ W + b), node-major."""
            fin = IN_DIM if li == 0 else HID_DIM
            fout = HID_DIM
            nk = fin // P
            nd = min(P, SHARD - d * P)
            wt, bt = w_tiles[li], b_tiles[li]
            ps = psd.tile([P, HID_DIM], _DT, tag="psd")
            for i, at in enumerate(ats):
                for k in range(nk):
                    nc.tensor.matmul(
                        ps[:nd, :fout],
                        lhsT=at[:, k * P:k * P + nd],
                        rhs=wt[:, k * fout:(k + 1) * fout],
                        start=(i == 0 and k == 0),
                        stop=False,
                    )
            nc.tensor.matmul(
                ps[:nd, :fout], lhsT=onesb[:, :nd], rhs=bt[:],
                start=False, stop=True,
            )
            hbt = hp.tile([P, HID_DIM], _DTG, tag="hb")
            nc.scalar.activation(
                hbt[:nd, :fout], ps[:nd, :fout],
                mybir.ActivationFunctionType.Tanh)
            return hbt

        def ag_half(hs_t, hf_t, half):
            if half == 0:
                ins_, outs_ = hs_t[:SA, :], hf_t[:N_CORES * SA, :]
            else:
                ins_, outs_ = hs_t[SA:, :], hf_t[N_CORES * SA:, :]
            nc.gpsimd.collective_compute(
                "AllGather",
                mybir.AluOpType.bypass,
                replica_groups=rg,
                ins=[ins_],
                outs=[outs_],
            )

        # ---- Layer 1: SpMM(x) -> dense W1 -> tanh -> hs1/AG ----
        # Postludes are emitted one block behind the SpMM matmuls so the PE
        # sequencer always has ready chunk-matmul work while a postlude
        # instruction parks on a cross-engine dependency (4-deep wait queue).
        def l1_post(d, at):
            nd = min(P, SHARD - d * P)
            hbt = dense(0, d, [at])
            nc.sync.dma_start(out=hs1[d * P: d * P + nd, :], in_=hbt[:nd, :])
            if d == 5:
                ag_half(hs1, hf1, 0)

        prev = None
        for d in range(N_BLOCKS):
            at = spmm_aggT(0, d)
            if prev is not None:
                l1_post(*prev)
            prev = (d, at)
        l1_post(*prev)
        ag_half(hs1, hf1, 1)

        # ---- Layer 2: SpMM(z1) -> dense W2 -> tanh -> z2T; L3 dense ----
        # Pass A: while AG half-1 is in flight, fully aggregate the first
        # K2 blocks' half-0 chunks into SBUF partials (frees PSUM + gather
        # tiles immediately, keeping the DMA engines fed through the
        # collective's latency).
        K2 = 4
        at0_l2 = [spmm_aggT(1, d, half=0) for d in range(K2)]

        def l2_post(d, ats):
            nd = min(P, SHARD - d * P)
            hbt = dense(1, d, ats)
            for k in range(HID_DIM // P):
                pt = pst.tile([P, P], _DTG, tag="pst")
                nc.tensor.transpose(
                    out=pt[:, :nd],
                    in_=hbt[:nd, k * P:(k + 1) * P],
                    identity=ident[:nd, :nd],
                )
                nc.vector.tensor_copy(
                    z2T[:, k * SHARD + d * P: k * SHARD + d * P + nd],
                    pt[:, :nd],
                )
            ps3 = psd.tile([P, HID_DIM], _DT, tag="psd")
            for k in range(HID_DIM // P):
                nc.tensor.matmul(
                    ps3[:nd, :OUT_DIM],
                    lhsT=z2T[:, k * SHARD + d * P: k * SHARD + d * P + nd],
                    rhs=w_tiles[2][:, k * OUT_DIM:(k + 1) * OUT_DIM],
                    start=(k == 0),
                    stop=(k == HID_DIM // P - 1),
                )
            hb3 = hp.tile([P, HID_DIM], _DTG, tag="hb")
            nc.scalar.activation(
                hb3[:nd, :OUT_DIM], ps3[:nd, :OUT_DIM],
                mybir.ActivationFunctionType.Copy)
            nc.sync.dma_start(
                out=hs3[d * P: d * P + nd, :], in_=hb3[:nd, :OUT_DIM])
            if d == 5:
                ag_half(hs3, hf3, 0)

        prev = None
        for d in range(N_BLOCKS):
            if d < K2:
                ats = [at0_l2[d], spmm_aggT(1, d, half=1)]
            else:
                ats = [spmm_aggT(1, d)]
            if prev is not None:
                l2_post(*prev)
            prev = (d, ats)
        l2_post(*prev)
        ag_half(hs3, hf3, 1)

        # ---- Layer 3: SpMM(h3) + b3 -> out ----
        def spmm3(d, half):
            """One accumulation group of L3's node-major SpMM in PSUM."""
            first, n = half_range(d, half)
            last = first + n - 1
            ranges = block_ranges(hf3, d)
            if half is not None:
                ranges = [ranges[half]]
            ps = psd.tile([P, HID_DIM], _DT, tag="psd")
            for ga, gb, gt in gathers(ranges, gidxA_t, OUT_DIM):
                for c in range(ga, gb):
                    nc.tensor.matmul(
                        ps[:, :OUT_DIM],
                        lhsT=s_all[:, c * P:(c + 1) * P],
                        rhs=gt[:, (c - ga) * OUT_DIM:(c - ga + 1) * OUT_DIM],
                        start=(c == first),
                        stop=(half == 0 and c == last),
                    )
            if half != 0:              # bias closes the group
                nc.tensor.matmul(
                    ps[:, :OUT_DIM], lhsT=onesb[:], rhs=b_tiles[2][:],
                    start=False, stop=True,
                )
            return ps

        # Pass A: half-0 partials for the first K3 blocks (fp32 in SBUF)
        K3 = 6
        ob0_l3 = []
        for d in range(K3):
            ps = spmm3(d, 0)
            ob0 = op.tile([P, OUT_DIM], _DT, tag="ob")
            nc.vector.tensor_copy(ob0[:], ps[:, :OUT_DIM])
            ob0_l3.append(ob0)
        for d in range(N_BLOCKS):
            nd = min(P, SHARD - d * P)
            ps = spmm3(d, 1 if d < K3 else None)
            ob = op.tile([P, OUT_DIM], _DT, tag="ob")
            if d < K3:
                nc.vector.scalar_tensor_tensor(
                    out=ob[:nd], in0=ps[:nd, :OUT_DIM], scalar=1.0,
                    in1=ob0_l3[d][:nd],
                    op0=mybir.AluOpType.mult, op1=mybir.AluOpType.add,
                )
            else:
                nc.vector.tensor_copy(ob[:nd], ps[:nd, :OUT_DIM])
            nc.sync.dma_start(out=out[d * P: d * P + nd, :], in_=ob[:nd])

    nc.compile()
    return nc


_CACHE = {}


def _get_kernel(schedule, nrep=1):
    key = (tuple(schedule), nrep)
    if key not in _CACHE:
        _CACHE[key] = _build(schedule, nrep)
    return _CACHE[key]


# ----------------------------------------------------------------------------
# Entry point
# ----------------------------------------------------------------------------

def kernel(x, W1, b1, W2, b2, W3, b3, edge_index, _trace=False, _trace_kwargs=None):
    x = np.asarray(x, dtype=np.float32)
    Ws = [np.ascontiguousarray(np.asarray(w, dtype=np.float32).astype(_BF))
          for w in (W1, W2, W3)]
    bs = [np.ascontiguousarray(
        np.asarray(b, dtype=np.float32).reshape(1, -1).astype(_BF))
        for b in (b1, b2, b3)]
    edge_index = np.asarray(edge_index)

    xg = np.ascontiguousarray(x.astype(_BF))
    schedule, gidx1_pc, gidxA_pc, s_pc = _preprocess(edge_index)
    nc = _get_kernel(schedule)

    in_maps = []
    for c in range(N_CORES):
        in_maps.append({
            "xg": xg,
            "W1": Ws[0], "W2": Ws[1], "W3": Ws[2],
            "b1": bs[0], "b2": bs[1], "b3": bs[2],
            "gidx1": gidx1_pc[c],
            "gidxA": gidxA_pc[c],
            "S": s_pc[c],
        })

    kwargs = {}
    if _trace:
        kwargs = {"trace": True, "trace_kwargs": _trace_kwargs or {}}
    try:
        res = run_bass_kernel_spmd(
            nc, in_maps, core_ids=list(range(N_CORES)), **kwargs)
    except Exception:
        # transient axon/device errors (e.g. NRT_EXEC_UNIT_UNRECOVERABLE on a
        # cold worker) clear on re-execution; retry once
        res = run_bass_kernel_spmd(
            nc, in_maps, core_ids=list(range(N_CORES)), **kwargs)
    out = np.concatenate([res.results[c]["out"] for c in range(N_CORES)], axis=0)
    if _trace:
        return out, res
    return out


# revision 26
# speedup vs baseline: 1.0928x; 1.0025x over previous
"""3-layer GCN (GCNConv x3, tanh between) on 8 Trainium2 NeuronCores.

Strategy (v2 — "SpMM-first" restructure of the node-range-sharded scheme):
  - GCN aggregation commutes with the dense transform (both linear), so
    layer 1 aggregates the *input features* x directly: each core holds
    the full bf16 copy of x in DRAM as a gather table, so layer-1 message
    gathering starts at t=0 with no AllGather and 256-wide (not 512-wide)
    rows. The aggregated block is then densified locally:
        z1 = tanh(aggT_x^T @ W1 + b1).
  - Layer 2 is also SpMM-first on the AllGathered z1 table. The chunk
    matmuls are emitted transposed (aggT_k = G_k^T @ S, [fin_chunk, dst])
    so the aggregate lands feature-major and feeds the dense matmul's
    stationary operand directly — no transposes for z1/z2-in.
  - Layer 3 is dense-first (h3 = z2 @ W3 is 256-wide, halving both its
    AllGather and its gather traffic); z2 is transposed on the
    TensorEngine (bf16, 1 cyc/row) to feature-major for that matmul.
  - Edges (+ one self-edge per node, weight dinv^2) are bucketed per
    128-dst-node block and *deduplicated by src* within the block
    (~10% fewer gathered rows and chunks); the per-chunk S matrix
    [row, dst_local] accumulates duplicate edge weights. One S table
    serves all three layers. Gathers use the GPSIMD dma_gather extended
    instruction batched 8 chunks / 1024 rows per instruction.
  - All matmuls are bf16 (1 cycle/row on the PE) accumulating fp32 in
    PSUM; biases enter PSUM as a rank-1 ones^T @ b matmul.

Numerics: messages, aggregates, and weights are bf16; accumulation is
fp32. End-to-end relative L2 error vs the fp32 reference is ~5e-3.
Host preprocessing touches only edge_index (sorting/bincount/unique),
the degree-derived edge weights, and bf16 casts of x/W/b.
"""
import sys

if "/opt/trn_rl_repo" not in sys.path:
    sys.path.insert(0, "/opt/trn_rl_repo")

from contextlib import ExitStack

import ml_dtypes
import numpy as np

import concourse.bass as bass
import concourse.bacc as bacc
import concourse.mybir as mybir
import concourse.tile as tile
from concourse.bass_utils import run_bass_kernel_spmd
from concourse.masks import make_identity

P = 128
N_CORES = 8
N_NODES = 10000
SHARD = N_NODES // N_CORES          # 1250
N_BLOCKS = (SHARD + P - 1) // P     # 10 (9 full + one 98-row block)
IN_DIM, HID_DIM, OUT_DIM = 256, 512, 256
SA, SB = 768, SHARD - 768           # split-AllGather half sizes
GK = 8                              # gather chunks per dma_gather instr

_DT = mybir.dt.float32
_DTG = mybir.dt.bfloat16

_BF = ml_dtypes.bfloat16


# ----------------------------------------------------------------------------
# Host-side edge preprocessing
# ----------------------------------------------------------------------------

def _preprocess(edge_index: np.ndarray):
    """Bucket edges by dst block, dedup srcs per block, build S + gidx.

    Within each block the deduped src rows are split by AllGather half
    (set0: q < SA, landing in hf[:8*SA]; set1: q >= SA) so the device can
    gather set0 rows as soon as AG half-0 completes, overlapping half-1's
    collective latency with gather traffic.

    Returns (schedule, gidx1_pc, gidxA_pc, s_pc):
      schedule    : (sched0, sched1) per-block chunk counts (all cores)
      gidx1_pc    : [P, C*8] int16 per core, plain node-id gather indices
                    (for the replicated x table)
      gidxA_pc    : [P, C*8] int16 per core, AG-layout indices; set1 rows
                    are RELATIVE to hf[8*SA:] (gathers use the half-table
                    slice as in_ap so deps attach per AG half)
      s_pc        : [P, C*P] bf16 per core, chunk-major S (dedup-accumulated
                    edge weights, S[row, dst_local])
    """
    src = np.asarray(edge_index[0], dtype=np.int64)
    dst = np.asarray(edge_index[1], dtype=np.int64)

    deg = (np.bincount(dst, minlength=N_NODES) + 1.0).astype(np.float32)
    dinv = (1.0 / np.sqrt(deg.astype(np.float64))).astype(np.float32)

    all_src = np.concatenate([src, np.arange(N_NODES, dtype=np.int64)])
    all_dst = np.concatenate([dst, np.arange(N_NODES, dtype=np.int64)])
    all_w = np.concatenate([dinv[src] * dinv[dst], dinv * dinv]).astype(np.float32)

    per_core = []
    n0 = np.zeros((N_CORES, N_BLOCKS), dtype=np.int64)
    n1 = np.zeros((N_CORES, N_BLOCKS), dtype=np.int64)
    for c in range(N_CORES):
        lo = c * SHARD
        mask = (all_dst >= lo) & (all_dst < lo + SHARD)
        csrc, cdst, cw = all_src[mask], all_dst[mask] - lo, all_w[mask]
        blocks = []
        for b in range(N_BLOCKS):
            bm = (cdst >= b * P) & (cdst < (b + 1) * P)
            bsrc, bdst, bw = csrc[bm], cdst[bm] - b * P, cw[bm]
            uniq, inv = np.unique(bsrc, return_inverse=True)
            in0 = (uniq % SHARD) < SA
            order = np.argsort(~in0, kind="stable")    # set0 rows first
            rank = np.empty(len(uniq), dtype=np.int64)
            rank[order] = np.arange(len(uniq))
            blocks.append((uniq[order], rank[inv], bdst, bw, int(in0.sum())))
            n0[c, b] = in0.sum()
            n1[c, b] = len(uniq) - in0.sum()
        per_core.append(blocks)

    sched0 = [int(x) for x in ((n0.max(axis=0) + P - 1) // P)]
    sched1 = [int(x) for x in ((n1.max(axis=0) + P - 1) // P)]
    stot = [a + b for a, b in zip(sched0, sched1)]
    C = sum(stot)
    cbases = np.concatenate([[0], np.cumsum(stot)])

    gidx1_pc, gidxA_pc, s_pc = [], [], []
    for c in range(N_CORES):
        flat = np.full(C * P, -1, dtype=np.int64)
        is1 = np.zeros(C * P, dtype=bool)
        S = np.zeros((C * P, P), dtype=np.float32)
        for b in range(N_BLOCKS):
            uniq, inv, bdst, bw, u0 = per_core[c][b]
            r0 = cbases[b] * P                     # set0 region
            r1 = r0 + sched0[b] * P                # set1 region
            is1[r1: r1 + sched1[b] * P] = True
            nu = len(uniq)
            pos = np.where(np.arange(nu) < u0,
                           r0 + np.arange(nu), r1 + np.arange(nu) - u0)
            flat[pos] = uniq
            np.add.at(S, (pos[inv], bdst), bw)
        pad = flat < 0
        flat_ids = np.where(pad, 0, flat)
        # Split-AllGather hfull layout:
        # node n = r*SHARD + q -> r*SA + q             (q < SA,  first half)
        #                      -> 8*SA + r*SB + (q-SA) (q >= SA, second half)
        r_, q_ = flat_ids // SHARD, flat_ids % SHARD
        ag = np.where(q_ < SA, r_ * SA + q_, 8 * SA + r_ * SB + (q_ - SA))
        ag = np.where(is1, ag - 8 * SA, ag)        # relative to half-1 slice
        ag = np.where(pad, 0, ag)                  # pads gather slice row 0

        # dma_gather int16 index layout: flat index i -> [i % 16, i // 16],
        # replicated across the 8 GPSIMD-core partition groups.
        def wrap(f):
            w = f.astype(np.int16).reshape(C * P // 16, 16).T
            return np.tile(w, (8, 1)).copy()

        gidx1_pc.append(wrap(flat_ids))
        gidxA_pc.append(wrap(ag))
        S2 = S.reshape(-1, P, P).transpose(1, 0, 2).reshape(P, -1)
        s_pc.append(np.ascontiguousarray(S2).astype(_BF))
    return (tuple(sched0), tuple(sched1)), gidx1_pc, gidxA_pc, s_pc


# ----------------------------------------------------------------------------
# Device kernel
# ----------------------------------------------------------------------------

def _build(schedule, nrep=1):
    sched0, sched1 = schedule
    stot = [a + b for a, b in zip(sched0, sched1)]
    C = sum(stot)
    nc = bacc.Bacc("TRN2", num_devices=N_CORES)

    xg = nc.dram_tensor("xg", [N_NODES, IN_DIM], _DTG, kind="ExternalInput")
    W1 = nc.dram_tensor("W1", [IN_DIM, HID_DIM], _DTG, kind="ExternalInput")
    W2 = nc.dram_tensor("W2", [HID_DIM, HID_DIM], _DTG, kind="ExternalInput")
    W3 = nc.dram_tensor("W3", [HID_DIM, OUT_DIM], _DTG, kind="ExternalInput")
    b1 = nc.dram_tensor("b1", [1, HID_DIM], _DTG, kind="ExternalInput")
    b2 = nc.dram_tensor("b2", [1, HID_DIM], _DTG, kind="ExternalInput")
    b3 = nc.dram_tensor("b3", [1, OUT_DIM], _DTG, kind="ExternalInput")
    gidx1 = nc.dram_tensor("gidx1", [P, C * 8], mybir.dt.int16, kind="ExternalInput")
    gidxA = nc.dram_tensor("gidxA", [P, C * 8], mybir.dt.int16, kind="ExternalInput")
    S = nc.dram_tensor("S", [P, C * P], _DTG, kind="ExternalInput")
    out = nc.dram_tensor("out", [SHARD, OUT_DIM], _DT, kind="ExternalOutput")

    hs1 = nc.dram_tensor("hs1", [SHARD, HID_DIM], _DTG)
    hs3 = nc.dram_tensor("hs3", [SHARD, OUT_DIM], _DTG)
    hf1 = nc.dram_tensor("hf1", [N_NODES, HID_DIM], _DTG, addr_space="Shared")
    hf3 = nc.dram_tensor("hf3", [N_NODES, OUT_DIM], _DTG, addr_space="Shared")

    rg = [list(range(N_CORES))]

    cbases = [0]
    for b in range(N_BLOCKS):
        cbases.append(cbases[-1] + stot[b])

    with tile.TileContext(nc) as tc, ExitStack() as ctx:
        const = ctx.enter_context(tc.tile_pool(name="const", bufs=1))
        gp = ctx.enter_context(tc.tile_pool(name="gather", bufs=6))
        ab = ctx.enter_context(tc.tile_pool(name="aggt", bufs=8))
        hp = ctx.enter_context(tc.tile_pool(name="hb", bufs=3))
        op = ctx.enter_context(tc.tile_pool(name="ob", bufs=8))
        psa = ctx.enter_context(tc.tile_pool(name="psa", bufs=4, space="PSUM"))
        psd = ctx.enter_context(tc.tile_pool(name="psd", bufs=2, space="PSUM"))
        pst = ctx.enter_context(tc.tile_pool(name="pst", bufs=2, space="PSUM"))

        ident = const.tile([P, P], _DTG)
        make_identity(nc, ident[:])
        onesb = const.tile([1, P], _DTG)
        nc.vector.memset(onesb[:], 1.0)

        # gather-critical loads first on the SP queue: gidx1, then S slices
        # (emitted per-block inside the L1 loop)
        gidx1_t = const.tile([P, C * 8], mybir.dt.int16)
        nc.sync.dma_start(out=gidx1_t[:], in_=gidx1[:])
        s_all = const.tile([P, C * P], _DTG)

        # weights / biases / gidxA on the Activation queue (not gather-
        # critical; keeps the SP queue free for S slices)
        gidxA_t = const.tile([P, C * 8], mybir.dt.int16)
        nc.scalar.dma_start(out=gidxA_t[:], in_=gidxA[:])

        w_tiles, b_tiles = [], []
        for W, b, fin, fout in [(W1, b1, IN_DIM, HID_DIM),
                                (W2, b2, HID_DIM, HID_DIM),
                                (W3, b3, HID_DIM, OUT_DIM)]:
            nk = fin // P
            wt = const.tile([P, nk * fout], _DTG, tag=f"w{fin}x{fout}")
            for k in range(nk):
                nc.scalar.dma_start(
                    out=wt[:].rearrange("p (k f) -> p k f", k=nk)[:, k:k + 1, :],
                    in_=W[:].rearrange("(k p) f -> p k f", p=P)[:, k:k + 1, :])
            bt = const.tile([1, fout], _DTG, tag=f"b{fout}")
            nc.scalar.dma_start(out=bt[:], in_=b[:])
            w_tiles.append(wt)
            b_tiles.append(bt)

        z2T = const.tile([P, (HID_DIM // P) * SHARD], _DTG)

        def gathers(ranges, gidx_t, fin):
            """Batched gathers over chunk ranges; returns [(ga, gb, gt)].

            ranges: list of (chunk_base, nchunks, in_ap); chunk indices are
            global stream positions. Each AG half is a separate in_ap slice
            so the gather's data dependency attaches to just that half.
            """
            tiles = []
            for cb, n, src in ranges:
                for g0 in range(0, n, GK):
                    g1 = min(g0 + GK, n)
                    n_sub = g1 - g0
                    gt = gp.tile([P, GK * HID_DIM], _DTG, tag="g")
                    nc.gpsimd.dma_gather(
                        out_ap=gt[:, :n_sub * fin].rearrange(
                            "p (c f) -> p c f", c=n_sub),
                        in_ap=src,
                        idxs_ap=gidx_t[:, (cb + g0) * 8: (cb + g1) * 8],
                        num_idxs=n_sub * P,
                        num_idxs_reg=n_sub * P,
                        elem_size=fin,
                    )
                    tiles.append((cb + g0, cb + g1, gt))
            return tiles

        def block_ranges(hf_t, d):
            """(chunk_base, n, in_ap) for dst-block d split by AG half."""
            return [
                (cbases[d], sched0[d], hf_t[:N_CORES * SA, :]),
                (cbases[d] + sched0[d], sched1[d], hf_t[N_CORES * SA:, :]),
            ]

        def half_range(d, half):
            """(first_chunk, nchunks) of dst-block d for AG half / both."""
            if half == 0:
                return cbases[d], sched0[d]
            if half == 1:
                return cbases[d] + sched0[d], sched1[d]
            return cbases[d], stot[d]

        def spmm_aggT(li, d, half=None):
            """Partial/full aggregate of dst-block d, transposed to bf16.

            k is the inner loop; each k accumulates in its OWN full PSUM
            bank (2 KiB zero region), so the interleaved groups can't stomp
            each other and gather tiles free after a single pass.
            half=0/1 aggregates only that AG half's chunks (partials are
            merged later in the dense matmul's accumulation).
            """
            fin = IN_DIM if li == 0 else HID_DIM
            nk = fin // P
            if li == 0:
                first, n = half_range(d, None)
                ranges = [(first, n, xg[:])]
                gidx_t = gidx1_t
            else:
                r0, r1 = block_ranges(hf1, d)
                ranges = [r0, r1] if half is None else [(r0, r1)[half]]
                first = ranges[0][0]
                n = sum(r[1] for r in ranges)
                gidx_t = gidxA_t
            last = first + n - 1
            tiles = gathers(ranges, gidx_t, fin)
            psk = [psa.tile([P, HID_DIM], _DT, tag="psa", name=f"psk{k}")
                   for k in range(nk)]
            for ga, gb, gt in tiles:
                for c in range(ga, gb):
                    for k in range(nk):
                        nc.tensor.matmul(
                            psk[k][:, :P],
                            lhsT=gt[:, (c - ga) * fin + k * P:
                                    (c - ga) * fin + (k + 1) * P],
                            rhs=s_all[:, c * P:(c + 1) * P],
                            start=(c == first),
                            stop=(c == last),
                        )
            at = ab.tile([P, 4 * P], _DTG, tag="at")
            for k in range(nk):
                nc.vector.tensor_copy(at[:, k * P:(k + 1) * P], psk[k][:, :P])
            return at

        def dense(li, d, ats):
            """z_{li+1} block d = tanh(sum_i ats[i]^T @ W + b), node-major."""
            fin = IN_DIM if li == 0 else HID_DIM
            fout = HID_DIM
            nk = fin // P
            nd = min(P, SHARD - d * P)
            wt, bt = w_tiles[li], b_tiles[li]
            ps = psd.tile([P, HID_DIM], _DT, tag="psd")
            for i, at in enumerate(ats):
                for k in range(nk):
                    nc.tensor.matmul(
                        ps[:nd, :fout],
                        lhsT=at[:, k * P:k * P + nd],
                        rhs=wt[:, k * fout:(k + 1) * fout],
                        start=(i == 0 and k == 0),
                        stop=False,
                    )
            nc.tensor.matmul(
                ps[:nd, :fout], lhsT=onesb[:, :nd], rhs=bt[:],
                start=False, stop=True,
            )
            hbt = hp.tile([P, HID_DIM], _DTG, tag="hb")
            nc.scalar.activation(
                hbt[:nd, :fout], ps[:nd, :fout],
                mybir.ActivationFunctionType.Tanh)
            return hbt

        def ag_half(hs_t, hf_t, half):
            if half == 0:
                ins_, outs_ = hs_t[:SA, :], hf_t[:N_CORES * SA, :]
            else:
                ins_, outs_ = hs_t[SA:, :], hf_t[N_CORES * SA:, :]
            nc.gpsimd.collective_compute(
                "AllGather",
                mybir.AluOpType.bypass,
                replica_groups=rg,
                ins=[ins_],
                outs=[outs_],
            )

        # ---- Layer 1: SpMM(x) -> dense W1 -> tanh -> hs1/AG ----
        # Postludes are emitted one block behind the SpMM matmuls so the PE
        # sequencer always has ready chunk-matmul work while a postlude
        # instruction parks on a cross-engine dependency (4-deep wait queue).
        def l1_post(d, at):
            nd = min(P, SHARD - d * P)
            hbt = dense(0, d, [at])
            nc.sync.dma_start(out=hs1[d * P: d * P + nd, :], in_=hbt[:nd, :])
            if d == 5:
                ag_half(hs1, hf1, 0)

        prev = None
        for d in range(N_BLOCKS):
            nc.sync.dma_start(
                out=s_all[:, cbases[d] * P: cbases[d + 1] * P],
                in_=S[:, cbases[d] * P: cbases[d + 1] * P])
            at = spmm_aggT(0, d)
            if prev is not None:
                l1_post(*prev)
            prev = (d, at)
        l1_post(*prev)
        ag_half(hs1, hf1, 1)

        # ---- Layer 2: SpMM(z1) -> dense W2 -> tanh -> z2T; L3 dense ----
        # Pass A: while AG half-1 is in flight, fully aggregate the first
        # K2 blocks' half-0 chunks into SBUF partials (frees PSUM + gather
        # tiles immediately, keeping the DMA engines fed through the
        # collective's latency).
        K2 = 4
        at0_l2 = [spmm_aggT(1, d, half=0) for d in range(K2)]

        def l2_post(d, ats):
            nd = min(P, SHARD - d * P)
            hbt = dense(1, d, ats)
            for k in range(HID_DIM // P):
                pt = pst.tile([P, P], _DTG, tag="pst")
                nc.tensor.transpose(
                    out=pt[:, :nd],
                    in_=hbt[:nd, k * P:(k + 1) * P],
                    identity=ident[:nd, :nd],
                )
                nc.vector.tensor_copy(
                    z2T[:, k * SHARD + d * P: k * SHARD + d * P + nd],
                    pt[:, :nd],
                )
            ps3 = psd.tile([P, HID_DIM], _DT, tag="psd")
            for k in range(HID_DIM // P):
                nc.tensor.matmul(
                    ps3[:nd, :OUT_DIM],
                    lhsT=z2T[:, k * SHARD + d * P: k * SHARD + d * P + nd],
                    rhs=w_tiles[2][:, k * OUT_DIM:(k + 1) * OUT_DIM],
                    start=(k == 0),
                    stop=(k == HID_DIM // P - 1),
                )
            hb3 = hp.tile([P, HID_DIM], _DTG, tag="hb")
            nc.scalar.activation(
                hb3[:nd, :OUT_DIM], ps3[:nd, :OUT_DIM],
                mybir.ActivationFunctionType.Copy)
            nc.sync.dma_start(
                out=hs3[d * P: d * P + nd, :], in_=hb3[:nd, :OUT_DIM])
            if d == 5:
                ag_half(hs3, hf3, 0)

        prev = None
        for d in range(N_BLOCKS):
            if d < K2:
                ats = [at0_l2[d], spmm_aggT(1, d, half=1)]
            else:
                ats = [spmm_aggT(1, d)]
            if prev is not None:
                l2_post(*prev)
            prev = (d, ats)
        l2_post(*prev)
        ag_half(hs3, hf3, 1)

        # ---- Layer 3: SpMM(h3) + b3 -> out ----
        def spmm3(d, half):
            """One accumulation group of L3's node-major SpMM in PSUM."""
            first, n = half_range(d, half)
            last = first + n - 1
            ranges = block_ranges(hf3, d)
            if half is not None:
                ranges = [ranges[half]]
            ps = psd.tile([P, HID_DIM], _DT, tag="psd")
            for ga, gb, gt in gathers(ranges, gidxA_t, OUT_DIM):
                for c in range(ga, gb):
                    nc.tensor.matmul(
                        ps[:, :OUT_DIM],
                        lhsT=s_all[:, c * P:(c + 1) * P],
                        rhs=gt[:, (c - ga) * OUT_DIM:(c - ga + 1) * OUT_DIM],
                        start=(c == first),
                        stop=(half == 0 and c == last),
                    )
            if half != 0:              # bias closes the group
                nc.tensor.matmul(
                    ps[:, :OUT_DIM], lhsT=onesb[:], rhs=b_tiles[2][:],
                    start=False, stop=True,
                )
            return ps

        # Pass A: half-0 partials for the first K3 blocks (fp32 in SBUF)
        K3 = 6
        ob0_l3 = []
        for d in range(K3):
            ps = spmm3(d, 0)
            ob0 = op.tile([P, OUT_DIM], _DT, tag="ob")
            nc.vector.tensor_copy(ob0[:], ps[:, :OUT_DIM])
            ob0_l3.append(ob0)
        for d in range(N_BLOCKS):
            nd = min(P, SHARD - d * P)
            ps = spmm3(d, 1 if d < K3 else None)
            ob = op.tile([P, OUT_DIM], _DT, tag="ob")
            if d < K3:
                nc.vector.scalar_tensor_tensor(
                    out=ob[:nd], in0=ps[:nd, :OUT_DIM], scalar=1.0,
                    in1=ob0_l3[d][:nd],
                    op0=mybir.AluOpType.mult, op1=mybir.AluOpType.add,
                )
            else:
                nc.vector.tensor_copy(ob[:nd], ps[:nd, :OUT_DIM])
            nc.sync.dma_start(out=out[d * P: d * P + nd, :], in_=ob[:nd])

    nc.compile()
    return nc


_CACHE = {}


def _get_kernel(schedule, nrep=1):
    key = (tuple(schedule), nrep)
    if key not in _CACHE:
        _CACHE[key] = _build(schedule, nrep)
    return _CACHE[key]


# ----------------------------------------------------------------------------
# Entry point
# ----------------------------------------------------------------------------

def kernel(x, W1, b1, W2, b2, W3, b3, edge_index, _trace=False, _trace_kwargs=None):
    x = np.asarray(x, dtype=np.float32)
    Ws = [np.ascontiguousarray(np.asarray(w, dtype=np.float32).astype(_BF))
          for w in (W1, W2, W3)]
    bs = [np.ascontiguousarray(
        np.asarray(b, dtype=np.float32).reshape(1, -1).astype(_BF))
        for b in (b1, b2, b3)]
    edge_index = np.asarray(edge_index)

    xg = np.ascontiguousarray(x.astype(_BF))
    schedule, gidx1_pc, gidxA_pc, s_pc = _preprocess(edge_index)
    nc = _get_kernel(schedule)

    in_maps = []
    for c in range(N_CORES):
        in_maps.append({
            "xg": xg,
            "W1": Ws[0], "W2": Ws[1], "W3": Ws[2],
            "b1": bs[0], "b2": bs[1], "b3": bs[2],
            "gidx1": gidx1_pc[c],
            "gidxA": gidxA_pc[c],
            "S": s_pc[c],
        })

    kwargs = {}
    if _trace:
        kwargs = {"trace": True, "trace_kwargs": _trace_kwargs or {}}
    try:
        res = run_bass_kernel_spmd(
            nc, in_maps, core_ids=list(range(N_CORES)), **kwargs)
    except Exception:
        # transient axon/device errors (e.g. NRT_EXEC_UNIT_UNRECOVERABLE on a
        # cold worker) clear on re-execution; retry once
        res = run_bass_kernel_spmd(
            nc, in_maps, core_ids=list(range(N_CORES)), **kwargs)
    out = np.concatenate([res.results[c]["out"] for c in range(N_CORES)], axis=0)
    if _trace:
        return out, res
    return out


# revision 27
# speedup vs baseline: 1.1228x; 1.0275x over previous
"""3-layer GCN (GCNConv x3, tanh between) on 8 Trainium2 NeuronCores.

Strategy (v2 — "SpMM-first" restructure of the node-range-sharded scheme):
  - GCN aggregation commutes with the dense transform (both linear), so
    layer 1 aggregates the *input features* x directly: each core holds
    the full bf16 copy of x in DRAM as a gather table, so layer-1 message
    gathering starts at t=0 with no AllGather and 256-wide (not 512-wide)
    rows. The aggregated block is then densified locally:
        z1 = tanh(aggT_x^T @ W1 + b1).
  - Layer 2 is also SpMM-first on the AllGathered z1 table. The chunk
    matmuls are emitted transposed (aggT_k = G_k^T @ S, [fin_chunk, dst])
    so the aggregate lands feature-major and feeds the dense matmul's
    stationary operand directly — no transposes for z1/z2-in.
  - Layer 3 is dense-first (h3 = z2 @ W3 is 256-wide, halving both its
    AllGather and its gather traffic); z2 is transposed on the
    TensorEngine (bf16, 1 cyc/row) to feature-major for that matmul.
  - Edges (+ one self-edge per node, weight dinv^2) are bucketed per
    128-dst-node block and *deduplicated by src* within the block
    (~10% fewer gathered rows and chunks); the per-chunk S matrix
    [row, dst_local] accumulates duplicate edge weights. One S table
    serves all three layers. Gathers use the GPSIMD dma_gather extended
    instruction batched 8 chunks / 1024 rows per instruction.
  - All matmuls are bf16 (1 cycle/row on the PE) accumulating fp32 in
    PSUM; biases enter PSUM as a rank-1 ones^T @ b matmul.

Numerics: messages, aggregates, and weights are bf16; accumulation is
fp32. End-to-end relative L2 error vs the fp32 reference is ~5e-3.
Host preprocessing touches only edge_index (sorting/bincount/unique),
the degree-derived edge weights, and bf16 casts of x/W/b.
"""
import sys

if "/opt/trn_rl_repo" not in sys.path:
    sys.path.insert(0, "/opt/trn_rl_repo")

from contextlib import ExitStack

import ml_dtypes
import numpy as np

import concourse.bass as bass
import concourse.bacc as bacc
import concourse.mybir as mybir
import concourse.tile as tile
from concourse.bass_utils import run_bass_kernel_spmd
from concourse.masks import make_identity

P = 128
N_CORES = 8
N_NODES = 10000
SHARD = N_NODES // N_CORES          # 1250
N_BLOCKS = (SHARD + P - 1) // P     # 10 (9 full + one 98-row block)
IN_DIM, HID_DIM, OUT_DIM = 256, 512, 256
SA, SB = 768, SHARD - 768           # split-AllGather half sizes
GK = 8                              # gather chunks per dma_gather instr

_DT = mybir.dt.float32
_DTG = mybir.dt.bfloat16

_BF = ml_dtypes.bfloat16


# ----------------------------------------------------------------------------
# Host-side edge preprocessing
# ----------------------------------------------------------------------------

def _preprocess(edge_index: np.ndarray):
    """Bucket edges by dst block, dedup srcs per block, build S + gidx.

    Within each block the deduped src rows are split by AllGather half
    (set0: q < SA, landing in hf[:8*SA]; set1: q >= SA) so the device can
    gather set0 rows as soon as AG half-0 completes, overlapping half-1's
    collective latency with gather traffic.

    Returns (schedule, gidx1_pc, gidxA_pc, s_pc):
      schedule    : (sched0, sched1) per-block chunk counts (all cores)
      gidx1_pc    : [P, C*8] int16 per core, plain node-id gather indices
                    (for the replicated x table)
      gidxA_pc    : [P, C*8] int16 per core, AG-layout indices; set1 rows
                    are RELATIVE to hf[8*SA:] (gathers use the half-table
                    slice as in_ap so deps attach per AG half)
      s_pc        : [P, C*P] bf16 per core, chunk-major S (dedup-accumulated
                    edge weights, S[row, dst_local])
    """
    src = np.asarray(edge_index[0], dtype=np.int64)
    dst = np.asarray(edge_index[1], dtype=np.int64)

    deg = (np.bincount(dst, minlength=N_NODES) + 1.0).astype(np.float32)
    dinv = (1.0 / np.sqrt(deg.astype(np.float64))).astype(np.float32)

    all_src = np.concatenate([src, np.arange(N_NODES, dtype=np.int64)])
    all_dst = np.concatenate([dst, np.arange(N_NODES, dtype=np.int64)])
    all_w = np.concatenate([dinv[src] * dinv[dst], dinv * dinv]).astype(np.float32)

    per_core = []
    n0 = np.zeros((N_CORES, N_BLOCKS), dtype=np.int64)
    n1 = np.zeros((N_CORES, N_BLOCKS), dtype=np.int64)
    for c in range(N_CORES):
        lo = c * SHARD
        mask = (all_dst >= lo) & (all_dst < lo + SHARD)
        csrc, cdst, cw = all_src[mask], all_dst[mask] - lo, all_w[mask]
        blocks = []
        for b in range(N_BLOCKS):
            bm = (cdst >= b * P) & (cdst < (b + 1) * P)
            bsrc, bdst, bw = csrc[bm], cdst[bm] - b * P, cw[bm]
            uniq, inv = np.unique(bsrc, return_inverse=True)
            in0 = (uniq % SHARD) < SA
            order = np.argsort(~in0, kind="stable")    # set0 rows first
            rank = np.empty(len(uniq), dtype=np.int64)
            rank[order] = np.arange(len(uniq))
            blocks.append((uniq[order], rank[inv], bdst, bw, int(in0.sum())))
            n0[c, b] = in0.sum()
            n1[c, b] = len(uniq) - in0.sum()
        per_core.append(blocks)

    sched0 = [int(x) for x in ((n0.max(axis=0) + P - 1) // P)]
    sched1 = [int(x) for x in ((n1.max(axis=0) + P - 1) // P)]
    stot = [a + b for a, b in zip(sched0, sched1)]
    C = sum(stot)
    cbases = np.concatenate([[0], np.cumsum(stot)])

    gidx1_pc, gidxA_pc, s_pc = [], [], []
    for c in range(N_CORES):
        flat = np.full(C * P, -1, dtype=np.int64)
        is1 = np.zeros(C * P, dtype=bool)
        S = np.zeros((C * P, P), dtype=np.float32)
        for b in range(N_BLOCKS):
            uniq, inv, bdst, bw, u0 = per_core[c][b]
            r0 = cbases[b] * P                     # set0 region
            r1 = r0 + sched0[b] * P                # set1 region
            is1[r1: r1 + sched1[b] * P] = True
            nu = len(uniq)
            pos = np.where(np.arange(nu) < u0,
                           r0 + np.arange(nu), r1 + np.arange(nu) - u0)
            flat[pos] = uniq
            np.add.at(S, (pos[inv], bdst), bw)
        pad = flat < 0
        flat_ids = np.where(pad, 0, flat)
        # Split-AllGather hfull layout:
        # node n = r*SHARD + q -> r*SA + q             (q < SA,  first half)
        #                      -> 8*SA + r*SB + (q-SA) (q >= SA, second half)
        r_, q_ = flat_ids // SHARD, flat_ids % SHARD
        ag = np.where(q_ < SA, r_ * SA + q_, 8 * SA + r_ * SB + (q_ - SA))
        ag = np.where(is1, ag - 8 * SA, ag)        # relative to half-1 slice
        ag = np.where(pad, 0, ag)                  # pads gather slice row 0

        # dma_gather int16 index layout: flat index i -> [i % 16, i // 16],
        # replicated across the 8 GPSIMD-core partition groups.
        def wrap(f):
            w = f.astype(np.int16).reshape(C * P // 16, 16).T
            return np.tile(w, (8, 1)).copy()

        gidx1_pc.append(wrap(flat_ids))
        gidxA_pc.append(wrap(ag))
        S2 = S.reshape(-1, P, P).transpose(1, 0, 2).reshape(P, -1)
        s_pc.append(np.ascontiguousarray(S2).astype(_BF))
    return (tuple(sched0), tuple(sched1)), gidx1_pc, gidxA_pc, s_pc


# ----------------------------------------------------------------------------
# Device kernel
# ----------------------------------------------------------------------------

def _build(schedule, nrep=1):
    sched0, sched1 = schedule
    stot = [a + b for a, b in zip(sched0, sched1)]
    C = sum(stot)
    nc = bacc.Bacc("TRN2", num_devices=N_CORES)

    xg = nc.dram_tensor("xg", [N_NODES, IN_DIM], _DTG, kind="ExternalInput")
    W1 = nc.dram_tensor("W1", [IN_DIM, HID_DIM], _DTG, kind="ExternalInput")
    W2 = nc.dram_tensor("W2", [HID_DIM, HID_DIM], _DTG, kind="ExternalInput")
    W3 = nc.dram_tensor("W3", [HID_DIM, OUT_DIM], _DTG, kind="ExternalInput")
    b1 = nc.dram_tensor("b1", [1, HID_DIM], _DTG, kind="ExternalInput")
    b2 = nc.dram_tensor("b2", [1, HID_DIM], _DTG, kind="ExternalInput")
    b3 = nc.dram_tensor("b3", [1, OUT_DIM], _DTG, kind="ExternalInput")
    gidx1 = nc.dram_tensor("gidx1", [P, C * 8], mybir.dt.int16, kind="ExternalInput")
    gidxA = nc.dram_tensor("gidxA", [P, C * 8], mybir.dt.int16, kind="ExternalInput")
    S = nc.dram_tensor("S", [P, C * P], _DTG, kind="ExternalInput")
    out = nc.dram_tensor("out", [SHARD, OUT_DIM], _DT, kind="ExternalOutput")

    hs1 = nc.dram_tensor("hs1", [SHARD, HID_DIM], _DTG)
    hs3 = nc.dram_tensor("hs3", [SHARD, OUT_DIM], _DTG)
    hf1 = nc.dram_tensor("hf1", [N_NODES, HID_DIM], _DTG, addr_space="Shared")
    hf3 = nc.dram_tensor("hf3", [N_NODES, OUT_DIM], _DTG, addr_space="Shared")

    rg = [list(range(N_CORES))]

    cbases = [0]
    for b in range(N_BLOCKS):
        cbases.append(cbases[-1] + stot[b])

    with tile.TileContext(nc) as tc, ExitStack() as ctx:
        const = ctx.enter_context(tc.tile_pool(name="const", bufs=1))
        gp = ctx.enter_context(tc.tile_pool(name="gather", bufs=6))
        ab = ctx.enter_context(tc.tile_pool(name="aggt", bufs=8))
        hp = ctx.enter_context(tc.tile_pool(name="hb", bufs=3))
        op = ctx.enter_context(tc.tile_pool(name="ob", bufs=8))
        psa = ctx.enter_context(tc.tile_pool(name="psa", bufs=4, space="PSUM"))
        psd = ctx.enter_context(tc.tile_pool(name="psd", bufs=2, space="PSUM"))
        pst = ctx.enter_context(tc.tile_pool(name="pst", bufs=2, space="PSUM"))

        ident = const.tile([P, P], _DTG)
        make_identity(nc, ident[:])
        onesb = const.tile([1, P], _DTG)
        nc.vector.memset(onesb[:], 1.0)

        # gather-critical loads first on the SP queue: gidx1, then S slices
        # (emitted per-block inside the L1 loop)
        gidx1_t = const.tile([P, C * 8], mybir.dt.int16)
        nc.sync.dma_start(out=gidx1_t[:], in_=gidx1[:])
        s_all = const.tile([P, C * P], _DTG)

        # weights / biases / gidxA on the Activation queue (not gather-
        # critical; keeps the SP queue free for S slices)
        gidxA_t = const.tile([P, C * 8], mybir.dt.int16)
        nc.scalar.dma_start(out=gidxA_t[:], in_=gidxA[:])

        w_tiles, b_tiles = [], []
        for W, b, fin, fout in [(W1, b1, IN_DIM, HID_DIM),
                                (W2, b2, HID_DIM, HID_DIM),
                                (W3, b3, HID_DIM, OUT_DIM)]:
            nk = fin // P
            wt = const.tile([P, nk * fout], _DTG, tag=f"w{fin}x{fout}")
            for k in range(nk):
                nc.scalar.dma_start(
                    out=wt[:].rearrange("p (k f) -> p k f", k=nk)[:, k:k + 1, :],
                    in_=W[:].rearrange("(k p) f -> p k f", p=P)[:, k:k + 1, :])
            bt = const.tile([1, fout], _DTG, tag=f"b{fout}")
            nc.scalar.dma_start(out=bt[:], in_=b[:])
            w_tiles.append(wt)
            b_tiles.append(bt)

        z2T = const.tile([P, (HID_DIM // P) * SHARD], _DTG)

        def gathers(ranges, gidx_t, fin):
            """Batched gathers over chunk ranges; returns [(ga, gb, gt)].

            ranges: list of (chunk_base, nchunks, in_ap); chunk indices are
            global stream positions. Each AG half is a separate in_ap slice
            so the gather's data dependency attaches to just that half.
            """
            tiles = []
            for cb, n, src in ranges:
                for g0 in range(0, n, GK):
                    g1 = min(g0 + GK, n)
                    n_sub = g1 - g0
                    gt = gp.tile([P, GK * HID_DIM], _DTG, tag="g")
                    nc.gpsimd.dma_gather(
                        out_ap=gt[:, :n_sub * fin].rearrange(
                            "p (c f) -> p c f", c=n_sub),
                        in_ap=src,
                        idxs_ap=gidx_t[:, (cb + g0) * 8: (cb + g1) * 8],
                        num_idxs=n_sub * P,
                        num_idxs_reg=n_sub * P,
                        elem_size=fin,
                    )
                    tiles.append((cb + g0, cb + g1, gt))
            return tiles

        def block_ranges(hf_t, d):
            """(chunk_base, n, in_ap) for dst-block d split by AG half."""
            return [
                (cbases[d], sched0[d], hf_t[:N_CORES * SA, :]),
                (cbases[d] + sched0[d], sched1[d], hf_t[N_CORES * SA:, :]),
            ]

        def half_range(d, half):
            """(first_chunk, nchunks) of dst-block d for AG half / both."""
            if half == 0:
                return cbases[d], sched0[d]
            if half == 1:
                return cbases[d] + sched0[d], sched1[d]
            return cbases[d], stot[d]

        def spmm_aggT(li, d, half=None):
            """Partial/full aggregate of dst-block d, transposed to bf16.

            k is the inner loop; each k accumulates in its OWN full PSUM
            bank (2 KiB zero region), so the interleaved groups can't stomp
            each other and gather tiles free after a single pass.
            half=0/1 aggregates only that AG half's chunks (partials are
            merged later in the dense matmul's accumulation).
            """
            fin = IN_DIM if li == 0 else HID_DIM
            nk = fin // P
            if li == 0:
                first, n = half_range(d, None)
                ranges = [(first, n, xg[:])]
                gidx_t = gidx1_t
            else:
                r0, r1 = block_ranges(hf1, d)
                ranges = [r0, r1] if half is None else [(r0, r1)[half]]
                first = ranges[0][0]
                n = sum(r[1] for r in ranges)
                gidx_t = gidxA_t
            last = first + n - 1
            tiles = gathers(ranges, gidx_t, fin)
            psk = [psa.tile([P, HID_DIM], _DT, tag="psa", name=f"psk{k}")
                   for k in range(nk)]
            for ga, gb, gt in tiles:
                for c in range(ga, gb):
                    for k in range(nk):
                        nc.tensor.matmul(
                            psk[k][:, :P],
                            lhsT=gt[:, (c - ga) * fin + k * P:
                                    (c - ga) * fin + (k + 1) * P],
                            rhs=s_all[:, c * P:(c + 1) * P],
                            start=(c == first),
                            stop=(c == last),
                        )
            at = ab.tile([P, 4 * P], _DTG, tag="at")
            for k in range(nk):
                nc.vector.tensor_copy(at[:, k * P:(k + 1) * P], psk[k][:, :P])
            return at

        def dense(li, d, ats):
            """z_{li+1} block d = tanh(sum_i ats[i]^T @ W + b), node-major."""
            fin = IN_DIM if li == 0 else HID_DIM
            fout = HID_DIM
            nk = fin // P
            nd = min(P, SHARD - d * P)
            wt, bt = w_tiles[li], b_tiles[li]
            ps = psd.tile([P, HID_DIM], _DT, tag="psd")
            for i, at in enumerate(ats):
                for k in range(nk):
                    nc.tensor.matmul(
                        ps[:nd, :fout],
                        lhsT=at[:, k * P:k * P + nd],
                        rhs=wt[:, k * fout:(k + 1) * fout],
                        start=(i == 0 and k == 0),
                        stop=False,
                    )
            nc.tensor.matmul(
                ps[:nd, :fout], lhsT=onesb[:, :nd], rhs=bt[:],
                start=False, stop=True,
            )
            hbt = hp.tile([P, HID_DIM], _DTG, tag="hb")
            nc.scalar.activation(
                hbt[:nd, :fout], ps[:nd, :fout],
                mybir.ActivationFunctionType.Tanh)
            return hbt

        def ag_half(hs_t, hf_t, half):
            if half == 0:
                ins_, outs_ = hs_t[:SA, :], hf_t[:N_CORES * SA, :]
            else:
                ins_, outs_ = hs_t[SA:, :], hf_t[N_CORES * SA:, :]
            nc.gpsimd.collective_compute(
                "AllGather",
                mybir.AluOpType.bypass,
                replica_groups=rg,
                ins=[ins_],
                outs=[outs_],
            )

        # ---- Layer 1: SpMM(x) -> dense W1 -> tanh -> hs1/AG ----
        # Postludes are emitted one block behind the SpMM matmuls so the PE
        # sequencer always has ready chunk-matmul work while a postlude
        # instruction parks on a cross-engine dependency (4-deep wait queue).
        def l1_post(d, at):
            nd = min(P, SHARD - d * P)
            hbt = dense(0, d, [at])
            nc.sync.dma_start(out=hs1[d * P: d * P + nd, :], in_=hbt[:nd, :])
            if d == 5:
                ag_half(hs1, hf1, 0)

        prev = None
        for d in range(N_BLOCKS):
            nc.sync.dma_start(
                out=s_all[:, cbases[d] * P: cbases[d + 1] * P],
                in_=S[:, cbases[d] * P: cbases[d + 1] * P])
            at = spmm_aggT(0, d)
            if prev is not None:
                l1_post(*prev)
            prev = (d, at)
        l1_post(*prev)
        ag_half(hs1, hf1, 1)

        # ---- Layer 2: SpMM(z1) -> dense W2 -> tanh -> z2T; L3 dense ----
        # Pass A: while AG half-1 is in flight, fully aggregate the first
        # K2 blocks' half-0 chunks into SBUF partials (frees PSUM + gather
        # tiles immediately, keeping the DMA engines fed through the
        # collective's latency).
        K2 = 5
        at0_l2 = [spmm_aggT(1, d, half=0) for d in range(K2)]

        def l2_post(d, ats):
            nd = min(P, SHARD - d * P)
            hbt = dense(1, d, ats)
            for k in range(HID_DIM // P):
                pt = pst.tile([P, P], _DTG, tag="pst")
                nc.tensor.transpose(
                    out=pt[:, :nd],
                    in_=hbt[:nd, k * P:(k + 1) * P],
                    identity=ident[:nd, :nd],
                )
                nc.vector.tensor_copy(
                    z2T[:, k * SHARD + d * P: k * SHARD + d * P + nd],
                    pt[:, :nd],
                )
            ps3 = psd.tile([P, HID_DIM], _DT, tag="psd")
            for k in range(HID_DIM // P):
                nc.tensor.matmul(
                    ps3[:nd, :OUT_DIM],
                    lhsT=z2T[:, k * SHARD + d * P: k * SHARD + d * P + nd],
                    rhs=w_tiles[2][:, k * OUT_DIM:(k + 1) * OUT_DIM],
                    start=(k == 0),
                    stop=(k == HID_DIM // P - 1),
                )
            hb3 = hp.tile([P, HID_DIM], _DTG, tag="hb")
            nc.scalar.activation(
                hb3[:nd, :OUT_DIM], ps3[:nd, :OUT_DIM],
                mybir.ActivationFunctionType.Copy)
            nc.sync.dma_start(
                out=hs3[d * P: d * P + nd, :], in_=hb3[:nd, :OUT_DIM])
            if d == 5:
                ag_half(hs3, hf3, 0)

        prev = None
        for d in range(N_BLOCKS):
            if d < K2:
                ats = [at0_l2[d], spmm_aggT(1, d, half=1)]
            else:
                ats = [spmm_aggT(1, d)]
            if prev is not None:
                l2_post(*prev)
            prev = (d, ats)
        l2_post(*prev)
        ag_half(hs3, hf3, 1)

        # ---- Layer 3: SpMM(h3) + b3 -> out ----
        def spmm3(d, half):
            """One accumulation group of L3's node-major SpMM in PSUM."""
            first, n = half_range(d, half)
            last = first + n - 1
            ranges = block_ranges(hf3, d)
            if half is not None:
                ranges = [ranges[half]]
            ps = psd.tile([P, HID_DIM], _DT, tag="psd")
            for ga, gb, gt in gathers(ranges, gidxA_t, OUT_DIM):
                for c in range(ga, gb):
                    nc.tensor.matmul(
                        ps[:, :OUT_DIM],
                        lhsT=s_all[:, c * P:(c + 1) * P],
                        rhs=gt[:, (c - ga) * OUT_DIM:(c - ga + 1) * OUT_DIM],
                        start=(c == first),
                        stop=(half == 0 and c == last),
                    )
            if half != 0:              # bias closes the group
                nc.tensor.matmul(
                    ps[:, :OUT_DIM], lhsT=onesb[:], rhs=b_tiles[2][:],
                    start=False, stop=True,
                )
            return ps

        # Pass A: half-0 partials for the first K3 blocks (fp32 in SBUF)
        K3 = 7
        ob0_l3 = []
        for d in range(K3):
            ps = spmm3(d, 0)
            ob0 = op.tile([P, OUT_DIM], _DT, tag="ob")
            nc.vector.tensor_copy(ob0[:], ps[:, :OUT_DIM])
            ob0_l3.append(ob0)
        for d in range(N_BLOCKS):
            nd = min(P, SHARD - d * P)
            ps = spmm3(d, 1 if d < K3 else None)
            ob = op.tile([P, OUT_DIM], _DT, tag="ob")
            if d < K3:
                nc.vector.scalar_tensor_tensor(
                    out=ob[:nd], in0=ps[:nd, :OUT_DIM], scalar=1.0,
                    in1=ob0_l3[d][:nd],
                    op0=mybir.AluOpType.mult, op1=mybir.AluOpType.add,
                )
            else:
                nc.vector.tensor_copy(ob[:nd], ps[:nd, :OUT_DIM])
            nc.sync.dma_start(out=out[d * P: d * P + nd, :], in_=ob[:nd])

    nc.compile()
    return nc


_CACHE = {}


def _get_kernel(schedule, nrep=1):
    key = (tuple(schedule), nrep)
    if key not in _CACHE:
        _CACHE[key] = _build(schedule, nrep)
    return _CACHE[key]


# ----------------------------------------------------------------------------
# Entry point
# ----------------------------------------------------------------------------

def kernel(x, W1, b1, W2, b2, W3, b3, edge_index, _trace=False, _trace_kwargs=None):
    x = np.asarray(x, dtype=np.float32)
    Ws = [np.ascontiguousarray(np.asarray(w, dtype=np.float32).astype(_BF))
          for w in (W1, W2, W3)]
    bs = [np.ascontiguousarray(
        np.asarray(b, dtype=np.float32).reshape(1, -1).astype(_BF))
        for b in (b1, b2, b3)]
    edge_index = np.asarray(edge_index)

    xg = np.ascontiguousarray(x.astype(_BF))
    schedule, gidx1_pc, gidxA_pc, s_pc = _preprocess(edge_index)
    nc = _get_kernel(schedule)

    in_maps = []
    for c in range(N_CORES):
        in_maps.append({
            "xg": xg,
            "W1": Ws[0], "W2": Ws[1], "W3": Ws[2],
            "b1": bs[0], "b2": bs[1], "b3": bs[2],
            "gidx1": gidx1_pc[c],
            "gidxA": gidxA_pc[c],
            "S": s_pc[c],
        })

    kwargs = {}
    if _trace:
        kwargs = {"trace": True, "trace_kwargs": _trace_kwargs or {}}
    try:
        res = run_bass_kernel_spmd(
            nc, in_maps, core_ids=list(range(N_CORES)), **kwargs)
    except Exception:
        # transient axon/device errors (e.g. NRT_EXEC_UNIT_UNRECOVERABLE on a
        # cold worker) clear on re-execution; retry once
        res = run_bass_kernel_spmd(
            nc, in_maps, core_ids=list(range(N_CORES)), **kwargs)
    out = np.concatenate([res.results[c]["out"] for c in range(N_CORES)], axis=0)
    if _trace:
        return out, res
    return out
